# revision 1
# baseline (speedup 1.0000x reference)
"""DTransformer forward on 8 trn2 NeuronCores (bass/Tile, single launch).

Sharding: core c handles batch b=c//4 and head pair p=c%4 (heads 2p, 2p+1)
of ALL three attention blocks. Phase 1 computes blocks 1&2 per-head
attention with the distance-decay bias (4 units/core). A per-batch
AllGather (groups [[0..3],[4..7]]) shares the per-head context in
transposed (feature-major) layout. Phase 2 rebuilds hq/ha (Wo + residual +
layernorm, duplicated inside the batch group) and runs block3 for the
core's 2 heads using the rank-1 structure of block3 scores (the
know_params query row is identical for every position, so the first
softmax reduces to prefix sums of one score vector). A second AllGather
shares block3 context; phase 3 produces a [512, 256] output shard
(row quarter q=c%4 of batch b): Wo3 + layernorm + the gated readout.

Device tricks: suffix tensor_tensor_scan for the decay tail (no
cancellation), ACT Exp/Sqrt with per-partition AP scale/bias, scores kept
fp16, dma_start_transpose (XBAR) for all 128x128 transposes, softmax
denominators via ACT accum_out, fp32r matmuls, ACT table-set grouping
(Exp/Sqrt batched over row-tile groups).
"""
import sys

if "/opt/trn_rl_repo" not in sys.path:
    sys.path.insert(0, "/opt/trn_rl_repo")

import numpy as np

import concourse.bacc as bacc
import concourse.mybir as mybir
import concourse.tile as tile
from concourse import bass_utils

dt = mybir.dt
AF = mybir.ActivationFunctionType
ALU = mybir.AluOpType
AX = mybir.AxisListType

S, D, H, DK, B = 2048, 256, 8, 32, 2
NT = S // 128
ISQ = float(1.0 / np.sqrt(DK))
GRP = 8
NEG = -30.0

_BUILT = {}


# --------------------------------------------------------------------------
# host-side input preparation (layout + parameter preprocessing only)
# --------------------------------------------------------------------------

def _softplus(x):
    return np.logaddexp(0.0, x)


def build_in_maps(inp):
    f32, f16 = np.float32, np.float16
    q_emb = np.asarray(inp["q_emb"], f32)
    qa_emb = np.asarray(inp["qa_emb"], f32)

    pos16 = np.maximum(
        np.arange(128)[:, None] + 2048 - np.arange(4096)[None, :], 0
    ).astype(f16)
    i_l = np.arange(128)[:, None]
    j_l = np.arange(128)[None, :]
    tri_pk = (j_l > i_l).astype(np.uint8)
    tri_st = (j_l >= i_l).astype(np.uint8)

    know = np.asarray(inp["know_params"], f32)[0, 0]
    q3 = know @ np.asarray(inp["b3_Wq"], f32) + np.asarray(inp["b3_bq"], f32)
    gam = {k: -_softplus(np.asarray(inp[k + "_gam"], f32)[:, 0, 0])
           for k in ("b1", "b2", "b3")}
    Wq = [np.asarray(inp["b1_Wq"], f32), np.asarray(inp["b2_Wq"], f32)]
    Wv = [np.asarray(inp["b1_Wv"], f32), np.asarray(inp["b2_Wv"], f32)]
    bq = [np.asarray(inp["b1_bq"], f32), np.asarray(inp["b2_bq"], f32)]
    bv = [np.asarray(inp["b1_bv"], f32), np.asarray(inp["b2_bv"], f32)]
    Wk3 = np.asarray(inp["b3_Wk"], f32)
    bk3 = np.asarray(inp["b3_bk"], f32)
    lvW = np.asarray(inp["lv_W"], f32)

    def chunk2(w):   # [256, F] -> [128, 2, F]
        return np.ascontiguousarray(w.reshape(2, 128, -1).transpose(1, 0, 2))

    def bc(v):       # [256] -> [128, 256] broadcast
        return np.broadcast_to(np.asarray(v, f32), (128, 256)).copy()

    lvw_pad = np.zeros((128, 8, 256), f16)
    for h in range(8):
        r0 = 32 * (h % 4)
        lvw_pad[r0:r0 + 32, h, :] = lvW.astype(f16)

    maps = []
    for c in range(8):
        b = c // 4
        p = c % 4
        heads = [2 * p, 2 * p + 1]
        X = [q_emb[b], qa_emb[b]]            # [2][2048, 256]

        xT = np.zeros((128, 2, 2, 2048), f32)
        wq_s = np.zeros((128, 2, 2, 2, 32), f32)
        wv_s = np.zeros((128, 2, 2, 64), f32)
        bq_col = np.zeros((32, 2, 2), f32)
        bv_bc = np.zeros((128, 2, 64), f32)
        gam_col = np.zeros((128, 2, 2), f32)
        for blk in range(2):
            xT[:, :, blk, :] = X[blk].T.reshape(2, 128, 2048).transpose(1, 0, 2)
            for hi, h in enumerate(heads):
                hs = slice(32 * h, 32 * h + 32)
                wq_s[:, :, blk, hi, :] = chunk2(Wq[blk][:, hs])
                bq_col[:, blk, hi] = bq[blk][hs]
                gam_col[:, blk, hi] = gam[("b1", "b2")[blk]][h]
            wv_s[:, :, blk, :] = chunk2(Wv[blk][:, 64 * p:64 * p + 64])
            bv_bc[:, blk, :] = bv[blk][64 * p:64 * p + 64]

        w16 = np.zeros((128, 2, 2), f16)
        ch_col = np.zeros((1, 2), f32)
        for hi, h in enumerate(heads):
            hs = slice(32 * h, 32 * h + 32)
            w = Wk3[:, hs] @ q3[hs]
            w16[:, :, hi] = w.reshape(2, 128).T.astype(f16)
            ch_col[0, hi] = float((bk3[hs] * q3[hs]).sum() * ISQ)

        qsel = np.zeros((128, 4), f32)
        qsel[:, p] = 1.0

        m = {
            "xT": xT,
            "wq_s": wq_s,
            "bq_col": bq_col,
            "wv_s": wv_s,
            "bv_bc": bv_bc,
            "gam_col": gam_col,
            "pos16": pos16,
            "tri_pk": tri_pk,
            "tri_st": tri_st,
            "r1": q_emb[b] + np.asarray(inp["b1_bo"], f32),
            "r2": qa_emb[b] + np.asarray(inp["b2_bo"], f32),
            "wo1": chunk2(np.asarray(inp["b1_Wo"], f32)),
            "wo2": chunk2(np.asarray(inp["b2_Wo"], f32)),
            "g1_bc": bc(inp["b1_lng"]), "b1_bc": bc(inp["b1_lnb"]),
            "g2_bc": bc(inp["b2_lng"]), "b2_bc": bc(inp["b2_lnb"]),
            "w16": w16,
            "ch_col": ch_col,
            "wv3_s16": chunk2(np.asarray(inp["b3_Wv"], f32)
                              [:, 64 * p:64 * p + 64]).astype(f16),
            "bv3_bc": np.broadcast_to(
                np.asarray(inp["b3_bv"], f32)[64 * p:64 * p + 64],
                (128, 64)).copy(),
            "gam3_col": np.broadcast_to(gam["b3"][heads], (128, 2)).copy(),
            "wo3": chunk2(np.asarray(inp["b3_Wo"], f32)),
            "res3_bc": bc(know + np.asarray(inp["b3_bo"], f32)),
            "g3_bc": bc(inp["b3_lng"]), "b3_bc": bc(inp["b3_lnb"]),
            "lvw_pad16": lvw_pad,
            "lvb_bc": bc(inp["lv_b"]),
            "qrT": np.ascontiguousarray(
                q_emb[b, 512 * p:512 * p + 512].T
                .reshape(2, 128, 512).transpose(1, 0, 2)),
            "kpT": np.ascontiguousarray(know.reshape(8, 32).T),
            "lkw": np.asarray(inp["lk_W"], f32),
            "lkb_col": np.ascontiguousarray(
                np.asarray(inp["lk_b"], f32).reshape(2, 128).T),
            "qsel_col": qsel,
        }
        maps.append(m)
    return maps


# --------------------------------------------------------------------------
# bass program
# --------------------------------------------------------------------------

def _ln(nc, pool, v, g_sb, b_sb, out, tag, eps):
    """out = layernorm(v) * g + b, v fp32 [128, 256]."""
    sv = pool.tile([128, 1], dt.float32, tag=f"sv{tag}")
    nc.vector.tensor_reduce(sv[:], v[:], axis=AX.X, op=ALU.add)
    sq = pool.tile([128, 256], dt.float32, tag=f"sq{tag}")
    s2v = pool.tile([128, 1], dt.float32, tag=f"s2v{tag}")
    nc.scalar.activation(sq[:], v[:], AF.Square, bias=0.0, scale=1.0,
                         accum_out=s2v[:])
    mu = pool.tile([128, 1], dt.float32, tag=f"mu{tag}")
    nc.vector.tensor_scalar(mu[:], sv[:], 1.0 / 256, None, op0=ALU.mult)
    mu2 = pool.tile([128, 1], dt.float32, tag=f"mu2{tag}")
    nc.vector.tensor_tensor(mu2[:], mu[:], mu[:], op=ALU.mult)
    var = pool.tile([128, 1], dt.float32, tag=f"var{tag}")
    nc.vector.scalar_tensor_tensor(var[:], s2v[:], 1.0 / 256, mu2[:],
                                   op0=ALU.mult, op1=ALU.subtract)
    sd = pool.tile([128, 1], dt.float32, tag=f"sd{tag}")
    nc.scalar.activation(sd[:], var[:], AF.Sqrt, bias=eps[:], scale=1.0)
    rstd = pool.tile([128, 1], dt.float32, tag=f"rstd{tag}")
    nc.vector.reciprocal(rstd[:], sd[:])
    xn = pool.tile([128, 256], dt.float32, tag=f"xn{tag}")
    nc.vector.tensor_scalar(xn[:], v[:], mu[:], rstd[:],
                            op0=ALU.subtract, op1=ALU.mult)
    nc.vector.tensor_tensor(xn[:], xn[:], g_sb[:], op=ALU.mult)
    nc.vector.tensor_tensor(out, xn[:], b_sb[:], op=ALU.add)


def _attn_triangle(nc, wp, ctxps, QT_ap_fn, sc_src, gam_ap, Vtile, v_off,
                   pos, tri, neg16, agdst, agrow, strict, sbc=None,
                   Pbc=None, ninvP=None, ones16=None):
    """The per-unit decay-bias attention triangle (16 row tiles, grouped).

    Blocks 1/2 path: sc_src(t, f, fw, psum_tile) emits the scores matmul;
    QT/e/suffix-scan pipeline. Block3 path (strict=True, sbc/Pbc given):
    rank-1 scores, no matmul/scan.
    """
    for g0 in range(0, NT, GRP):
        tiles = list(range(g0, min(g0 + GRP, NT)))
        sc_sb, chain = {}, {}
        if not strict:
            # stage A: scores -> e (Exp), fp16 score copy
            for t in tiles:
                W = 128 * (t + 1)
                sc_sb[t] = wp.tile([128, 2048], dt.float16, tag=f"sc{t - g0}", name="scsb")
                e = wp.tile([128, 2048], dt.float16, tag=f"ch{t - g0}", name="e")
                chain[t] = e
                sc_src(t, e, sc_sb[t])
                nc.vector.copy_predicated(e[:, W - 128:W], tri[:], neg16[1])
            # stage B: suffix scan -> tail, u = max(tail,0)*pos
            invZ = wp.tile([128, GRP], dt.float32, tag="invZ")
            usb = {}
            for t in tiles:
                W = 128 * (t + 1)
                suf = wp.tile([128, 2049], dt.float16, tag=f"sf{t - g0}", name="suf")
                nc.vector.memset(suf[:, W:W + 1], 0.0)
                nc.vector.tensor_tensor_scan(
                    suf[:, :W][:, ::-1], chain[t][:, :W][:, ::-1],
                    chain[t][:, :W][:, ::-1], 0.0, op0=ALU.add, op1=ALU.bypass)
                nc.vector.reciprocal(invZ[:, t - g0:t - g0 + 1], suf[:, 0:1])
                u = wp.tile([128, 2048], dt.float16, tag=f"ch{t - g0}", name="u")
                nc.vector.scalar_tensor_tensor(
                    u[:, :W], suf[:, 1:W + 1], 0.0,
                    pos[:, 2048 - 128 * t:2048 - 128 * t + W],
                    op0=ALU.max, op1=ALU.mult)
                usb[t] = u
        else:
            # block3 stage B': t1 = P*(-1/Pprev) + 1, u = max(t1,0)*pos
            usb = {}
            invZ = None
            for t in tiles:
                W = 128 * (t + 1)
                t1 = wp.tile([128, 2048], dt.float16, tag=f"sf{t - g0}", name="t1")
                nc.vector.scalar_tensor_tensor(
                    t1[:, :W], Pbc[:, :W], ninvP[:, t:t + 1], ones16[:, :W],
                    op0=ALU.mult, op1=ALU.add)
                u = wp.tile([128, 2048], dt.float16, tag=f"ch{t - g0}", name="u")
                nc.vector.scalar_tensor_tensor(
                    u[:, :W], t1[:, :W], 0.0,
                    pos[:, 2048 - 128 * t:2048 - 128 * t + W],
                    op0=ALU.max, op1=ALU.mult)
                usb[t] = u
        # stage C: r = sqrt(u * invZ)  (invZ=1 for block3)
        rsb = {}
        for t in tiles:
            W = 128 * (t + 1)
            r = wp.tile([128, 2048], dt.float16, tag=f"sf{t - g0}", name="r")
            if strict:
                nc.scalar.activation(r[:, :W], usb[t][:, :W], AF.Sqrt,
                                     bias=0.0, scale=1.0)
            else:
                nc.scalar.activation(r[:, :W], usb[t][:, :W], AF.Sqrt,
                                     bias=0.0, scale=invZ[:, t - g0:t - g0 + 1])
            rsb[t] = r
        # stage D: eff, scores2, e2, p2, transpose, ctx matmul, store
        for t in tiles:
            W = 128 * (t + 1)
            m0 = wp.tile([128, 2048], dt.float16, tag=f"ch{t - g0}", name="m0")
            nc.scalar.activation(m0[:, :W], rsb[t][:, :W], AF.Exp,
                                 bias=0.0, scale=gam_ap)
            s2 = wp.tile([128, 2048], dt.float16, tag=f"sf{t - g0}", name="s2")
            src2 = sbc[:, :W] if strict else sc_sb[t][:, :W]
            nc.vector.scalar_tensor_tensor(
                s2[:, :W], m0[:, :W], 1e-5, src2, op0=ALU.max, op1=ALU.mult)
            nc.vector.copy_predicated(s2[:, W - 128:W], tri[:], neg16[0])
            e2 = wp.tile([128, 2048], dt.float16, tag=f"ch{t - g0}", name="e2")
            Z2 = wp.tile([128, 1], dt.float32, tag="Z2")
            nc.scalar.activation(e2[:, :W], s2[:, :W], AF.Exp, bias=0.0,
                                 scale=1.0, accum_out=Z2[:])
            iZ2 = wp.tile([128, 1], dt.float32, tag="iZ2")
            nc.vector.reciprocal(iZ2[:], Z2[:])
            p2 = wp.tile([128, 2048], dt.float16, tag=f"sf{t - g0}", name="p2")
            nc.vector.tensor_scalar(p2[:, :W], e2[:, :W], iZ2[:], None,
                                    op0=ALU.mult)
            if strict and t == 0:
                nc.vector.memset(p2[0:1, 0:128], 0.0)
            p2T = wp.tile([128, 16, 128], dt.float16, tag=f"ch{t - g0}", name="p2T")
            nc.sync.dma_start_transpose(p2T[:, :t + 1, :], p2[:, :W])
            cps = ctxps.tile([32, 128], dt.float32, tag="ctx")
            for jb in range(t + 1):
                nc.tensor.matmul(cps[:], Vtile[:, jb, v_off:v_off + 32],
                                 p2T[:, jb, :], start=(jb == 0), stop=(jb == t))
            csb = wp.tile([32, 128], dt.float32, tag="csb", name="csb")
            nc.vector.tensor_copy(csb[:], cps[:])
            nc.sync.dma_start(
                agdst[agrow:agrow + 32, 128 * t:128 * t + 128], csb[:])


def build_bass(sim=False):
    nc = bacc.Bacc("TRN2", target_bir_lowering=False)

    def din(name, shape, dtyp=dt.float32):
        return nc.dram_tensor(name, shape, dtyp, kind="ExternalInput")

    tin = {
        "xT": din("xT", [128, 2, 2, 2048], dt.float32r),
        "wq_s": din("wq_s", [128, 2, 2, 2, 32], dt.float32r),
        "bq_col": din("bq_col", [32, 2, 2]),
        "wv_s": din("wv_s", [128, 2, 2, 64], dt.float32r),
        "bv_bc": din("bv_bc", [128, 2, 64]),
        "gam_col": din("gam_col", [128, 2, 2]),
        "pos16": din("pos16", [128, 4096], dt.float16),
        "tri_pk": din("tri_pk", [128, 128], dt.uint8),
        "tri_st": din("tri_st", [128, 128], dt.uint8),
        "r1": din("r1", [2048, 256]),
        "r2": din("r2", [2048, 256]),
        "wo1": din("wo1", [128, 2, 256], dt.float32r),
        "wo2": din("wo2", [128, 2, 256], dt.float32r),
        "g1_bc": din("g1_bc", [128, 256]), "b1_bc": din("b1_bc", [128, 256]),
        "g2_bc": din("g2_bc", [128, 256]), "b2_bc": din("b2_bc", [128, 256]),
        "w16": din("w16", [128, 2, 2], dt.float16),
        "ch_col": din("ch_col", [1, 2]),
        "wv3_s16": din("wv3_s16", [128, 2, 64], dt.float16),
        "bv3_bc": din("bv3_bc", [128, 64]),
        "gam3_col": din("gam3_col", [128, 2]),
        "wo3": din("wo3", [128, 2, 256]),
        "res3_bc": din("res3_bc", [128, 256]),
        "g3_bc": din("g3_bc", [128, 256]), "b3_bc": din("b3_bc", [128, 256]),
        "lvw_pad16": din("lvw_pad16", [128, 8, 256], dt.float16),
        "lvb_bc": din("lvb_bc", [128, 256]),
        "qrT": din("qrT", [128, 2, 512]),
        "kpT": din("kpT", [32, 8]),
        "lkw": din("lkw", [32, 256]),
        "lkb_col": din("lkb_col", [128, 2]),
        "qsel_col": din("qsel_col", [128, 4]),
    }
    out_t = nc.dram_tensor("out", [512, 256], dt.float32,
                           kind="ExternalOutput")

    with tile.TileContext(nc) as tc, \
         tc.tile_pool(name="consts", bufs=1) as cs, \
         tc.tile_pool(name="dram", bufs=1, space="DRAM") as dram:
        pos = cs.tile([128, 4096], dt.float16)
        nc.sync.dma_start(pos[:], tin["pos16"][:])
        tpk = cs.tile([128, 128], dt.uint8)
        nc.sync.dma_start(tpk[:], tin["tri_pk"][:])
        tst = cs.tile([128, 128], dt.uint8)
        nc.sync.dma_start(tst[:], tin["tri_st"][:])
        gamc = cs.tile([128, 2, 2], dt.float32)
        nc.sync.dma_start(gamc[:], tin["gam_col"][:])
        gam3c = cs.tile([128, 2], dt.float32)
        nc.sync.dma_start(gam3c[:], tin["gam3_col"][:])
        bqc = cs.tile([32, 2, 2], dt.float32)
        nc.sync.dma_start(bqc[:], tin["bq_col"][:])
        bvb = cs.tile([128, 2, 64], dt.float32)
        nc.sync.dma_start(bvb[:], tin["bv_bc"][:])
        neg16 = cs.tile([128, 128], dt.float16)
        nc.vector.memset(neg16[:], NEG)
        zero16 = cs.tile([128, 128], dt.float16)
        nc.vector.memset(zero16[:], 0.0)
        ones16 = cs.tile([128, 2048], dt.float16)
        nc.vector.memset(ones16[:], 1.0)
        tch = cs.tile([1, 4], dt.float16)
        nc.vector.tensor_copy(tch[:, 0:1], pos[:1, :1])
        epsc = cs.tile([128, 1], dt.float32)
        nc.vector.memset(epsc[:], 1e-5)

        agin1 = dram.tile([128, 2048], dt.float32)
        agout1 = dram.tile([512, 2048], dt.float32)
        agin2 = dram.tile([64, 2048], dt.float32)
        agout2 = dram.tile([256, 2048], dt.float32)
        pbuf = dram.tile([2, 2049], dt.float32)

        # ---------------- phase 1 ----------------
        QT = cs.tile([32, 2, 2, 2048], dt.float16)
        V16 = cs.tile([128, 2, 16, 64], dt.float16)
        with tc.tile_pool(name="proj", bufs=1) as pj, \
             tc.tile_pool(name="pjps", bufs=2, space="PSUM") as pjps:
            xTs = pj.tile([128, 2, 2, 2048], dt.float32r)
            nc.sync.dma_start(xTs[:], tin["xT"][:])
            wqs = pj.tile([128, 2, 2, 2, 32], dt.float32r)
            nc.sync.dma_start(wqs[:], tin["wq_s"][:])
            wvs = pj.tile([128, 2, 2, 64], dt.float32r)
            nc.sync.dma_start(wvs[:], tin["wv_s"][:])
            for blk in range(2):
                for hi in range(2):
                    for f in range(4):
                        ps = pjps.tile([32, 512], dt.float32, tag="qt")
                        for cch in range(2):
                            nc.tensor.matmul(
                                ps[:], wqs[:, cch, blk, hi, :],
                                xTs[:, cch, blk, 512 * f:512 * f + 512],
                                start=(cch == 0), stop=(cch == 1))
                        nc.scalar.activation(
                            QT[:, blk, hi, 512 * f:512 * f + 512], ps[:],
                            AF.Identity, bias=bqc[:, blk, hi:hi + 1],
                            scale=1.0)
                for jb in range(16):
                    ps = pjps.tile([128, 64], dt.float32, tag="v")
                    for cch in range(2):
                        nc.tensor.matmul(
                            ps[:], xTs[:, cch, blk, 128 * jb:128 * jb + 128],
                            wvs[:, cch, blk, :], start=(cch == 0),
                            stop=(cch == 1))
                    nc.vector.tensor_tensor(V16[:, blk, jb, :], ps[:],
                                            bvb[:, blk, :], op=ALU.add)

        with tc.tile_pool(name="p1", bufs=1) as wp, \
             tc.tile_pool(name="scps", bufs=3, space="PSUM") as scps, \
             tc.tile_pool(name="ctxps", bufs=2, space="PSUM") as ctxps:
            for blk in range(2):
                for hi in range(2):
                    def sc_src(t, e_tile, sc16_tile, blk=blk, hi=hi):
                        W = 128 * (t + 1)
                        for f in range((W + 511) // 512):
                            fw = min(512, W - 512 * f)
                            ps = scps.tile([128, 512], dt.float32, tag="sc")
                            nc.tensor.matmul(
                                ps[:, :fw],
                                QT[:, blk, hi, 128 * t:128 * t + 128],
                                QT[:, blk, hi, 512 * f:512 * f + fw],
                                start=True, stop=True)
                            nc.scalar.activation(
                                e_tile[:, 512 * f:512 * f + fw], ps[:, :fw],
                                AF.Exp, bias=0.0, scale=ISQ)
                            nc.vector.tensor_scalar(
                                sc16_tile[:, 512 * f:512 * f + fw],
                                ps[:, :fw], ISQ, None, op0=ALU.mult)
                    _attn_triangle(
                        nc, wp, ctxps, None, sc_src,
                        gamc[:, blk, hi:hi + 1], V16[:, blk], 32 * hi,
                        pos, tpk, (neg16, zero16), agin1,
                        64 * blk + 32 * hi, strict=False)

        if sim:
            for rr in range(4):
                nc.gpsimd.dma_start(agout1[128 * rr:128 * rr + 128, :],
                                    agin1[:])
        else:
            nc.gpsimd.collective_compute(
                "AllGather", ALU.bypass, ins=[agin1.opt()],
                outs=[agout1.opt()],
                replica_groups=[[0, 1, 2, 3], [4, 5, 6, 7]])

        # ---------------- phase 2: hq/ha ----------------
        hq16 = cs.tile([128, 2, 2048], dt.float16)
        ha16 = cs.tile([128, 2, 2048], dt.float16)
        with tc.tile_pool(name="p2", bufs=1) as p2, \
             tc.tile_pool(name="p2ps", bufs=2, space="PSUM") as p2ps:
            for which, (wo_n, res_n, g_n, bb_n, dstT) in enumerate([
                    ("wo1", "r1", "g1_bc", "b1_bc", hq16),
                    ("wo2", "r2", "g2_bc", "b2_bc", ha16)]):
                wo_sb = p2.tile([128, 2, 256], dt.float32r, tag="wo")
                nc.sync.dma_start(wo_sb[:], tin[wo_n][:])
                g_sb = p2.tile([128, 256], dt.float32, tag="g")
                nc.sync.dma_start(g_sb[:], tin[g_n][:])
                bb_sb = p2.tile([128, 256], dt.float32, tag="bb")
                nc.sync.dma_start(bb_sb[:], tin[bb_n][:])
                ctxT = p2.tile([128, 2, 2048], dt.float32r, tag="ctxT")
                for cch in range(2):
                    nc.gpsimd.dma_start(
                        ctxT[0:64, cch, :],
                        agout1[256 * cch + 64 * which:
                               256 * cch + 64 * which + 64, :])
                    nc.gpsimd.dma_start(
                        ctxT[64:128, cch, :],
                        agout1[256 * cch + 128 + 64 * which:
                               256 * cch + 128 + 64 * which + 64, :])
                h16 = p2.tile([128, 16, 256], dt.float16, tag="h16")
                for ic in range(16):
                    ps = p2ps.tile([128, 256], dt.float32, tag="wops")
                    for cch in range(2):
                        nc.tensor.matmul(
                            ps[:], ctxT[:, cch, 128 * ic:128 * ic + 128],
                            wo_sb[:, cch, :], start=(cch == 0),
                            stop=(cch == 1))
                    res_sb = p2.tile([128, 256], dt.float32, tag="res")
                    nc.sync.dma_start(res_sb[:],
                                      tin[res_n][128 * ic:128 * ic + 128, :])
                    v = p2.tile([128, 256], dt.float32, tag="v")
                    nc.vector.tensor_tensor(v[:], ps[:], res_sb[:], op=ALU.add)
                    _ln(nc, p2, v, g_sb, bb_sb, h16[:, ic, :], "a", epsc)
                for ic in range(16):
                    nc.sync.dma_start_transpose(
                        dstT[:, :, 128 * ic:128 * ic + 128], h16[:, ic, :])

        # ---------------- phase 2: block3 ----------------
        with tc.tile_pool(name="b3", bufs=1) as wp, \
             tc.tile_pool(name="b3ps", bufs=2, space="PSUM") as ps3, \
             tc.tile_pool(name="b3ctx", bufs=2, space="PSUM") as ctxps:
            w16sb = wp.tile([128, 2, 2], dt.float16, tag="w16")
            nc.sync.dma_start(w16sb[:], tin["w16"][:])
            chc = wp.tile([1, 2], dt.float32, tag="chc")
            nc.sync.dma_start(chc[:], tin["ch_col"][:])
            wv3 = wp.tile([128, 2, 64], dt.float16, tag="wv3")
            nc.sync.dma_start(wv3[:], tin["wv3_s16"][:])
            bv3 = wp.tile([128, 64], dt.float32, tag="bv3")
            nc.sync.dma_start(bv3[:], tin["bv3_bc"][:])
            one1 = wp.tile([1, 1], dt.float32, tag="one1")
            nc.vector.memset(one1[:], 1.0)
            onesrow = wp.tile([1, 128], dt.float32, tag="onesrow")
            nc.vector.memset(onesrow[:], 1.0)

            V3 = wp.tile([128, 16, 64], dt.float16, tag="V3")
            for jb in range(16):
                vps = ps3.tile([128, 64], dt.float32, tag="v3")
                for cch in range(2):
                    nc.tensor.matmul(
                        vps[:], ha16[:, cch, 128 * jb:128 * jb + 128],
                        wv3[:, cch, :], start=(cch == 0), stop=(cch == 1))
                nc.vector.tensor_tensor(V3[:, jb, :], vps[:], bv3[:],
                                        op=ALU.add)

            for hi in range(2):
                s_row = wp.tile([1, 2048], dt.float32, tag="srow")
                for f in range(4):
                    sp = ps3.tile([1, 512], dt.float32, tag="s")
                    for cch in range(2):
                        nc.tensor.matmul(
                            sp[:], w16sb[:, cch, hi:hi + 1],
                            hq16[:, cch, 512 * f:512 * f + 512],
                            start=(cch == 0), stop=(cch == 1))
                    nc.scalar.activation(
                        s_row[:, 512 * f:512 * f + 512], sp[:], AF.Identity,
                        bias=chc[:, hi:hi + 1], scale=ISQ)
                smax = wp.tile([1, 1], dt.float32, tag="smax")
                nc.vector.tensor_reduce(smax[:], s_row[:], axis=AX.X,
                                        op=ALU.max)
                nsmax = wp.tile([1, 1], dt.float32, tag="nsmax")
                nc.vector.tensor_scalar(nsmax[:], smax[:], -1.0, None,
                                        op0=ALU.mult)
                e3 = wp.tile([1, 2048], dt.float32, tag="e3")
                nc.scalar.activation(e3[:], s_row[:], AF.Exp, bias=nsmax[:],
                                     scale=1.0)
                P_row = wp.tile([1, 2048], dt.float32, tag="Prow")
                nc.vector.tensor_tensor_scan(P_row[:], e3[:], e3[:], 0.0,
                                             op0=ALU.add, op1=ALU.bypass)
                nc.sync.dma_start(pbuf[hi, 0:1], one1[:])
                nc.sync.dma_start(pbuf[hi, 1:2049], P_row[:])
                npcol = wp.tile([128, 16], dt.float32, tag="npcol")
                pcol = wp.tile([128, 16], dt.float32, tag="pcol")
                nc.sync.dma_start(
                    pcol[:], pbuf[hi, 0:2048].rearrange("(t p) -> p t", p=128))
                nc.vector.tensor_scalar(pcol[:], pcol[:], -1.0, None,
                                        op0=ALU.mult)
                nc.vector.reciprocal(npcol[:], pcol[:])
                P_bc = wp.tile([128, 2048], dt.float32, tag="Pbc")
                s_bc = wp.tile([128, 2048], dt.float16, tag="sbc")
                for f in range(4):
                    bp = ps3.tile([128, 512], dt.float32, tag="bc")
                    nc.tensor.matmul(bp[:], onesrow[:],
                                     P_row[:, 512 * f:512 * f + 512],
                                     start=True, stop=True)
                    nc.vector.tensor_copy(P_bc[:, 512 * f:512 * f + 512],
                                          bp[:])
                    bs = ps3.tile([128, 512], dt.float32, tag="bc")
                    nc.tensor.matmul(bs[:], onesrow[:],
                                     s_row[:, 512 * f:512 * f + 512],
                                     start=True, stop=True)
                    nc.vector.tensor_copy(s_bc[:, 512 * f:512 * f + 512],
                                          bs[:])
                _attn_triangle(
                    nc, wp, ctxps, None, None, gam3c[:, hi:hi + 1],
                    V3, 32 * hi, pos, tst, (neg16, zero16), agin2, 32 * hi,
                    strict=True, sbc=s_bc, Pbc=P_bc, ninvP=npcol,
                    ones16=ones16)

        if sim:
            for rr in range(4):
                nc.gpsimd.dma_start(agout2[64 * rr:64 * rr + 64, :],
                                    agin2[:])
        else:
            nc.gpsimd.collective_compute(
                "AllGather", ALU.bypass, ins=[agin2.opt()],
                outs=[agout2.opt()],
                replica_groups=[[0, 1, 2, 3], [4, 5, 6, 7]])

        # ---------------- phase 3 ----------------
        with tc.tile_pool(name="p3", bufs=1) as wp, \
             tc.tile_pool(name="p3ps", bufs=2, space="PSUM") as ps:
            wo3 = wp.tile([128, 2, 256], dt.float32, tag="wo3")
            nc.sync.dma_start(wo3[:], tin["wo3"][:])
            res3 = wp.tile([128, 256], dt.float32, tag="res3")
            nc.sync.dma_start(res3[:], tin["res3_bc"][:])
            g3 = wp.tile([128, 256], dt.float32, tag="g3")
            nc.sync.dma_start(g3[:], tin["g3_bc"][:])
            b3 = wp.tile([128, 256], dt.float32, tag="b3")
            nc.sync.dma_start(b3[:], tin["b3_bc"][:])
            lvw = wp.tile([128, 8, 256], dt.float16, tag="lvw")
            nc.sync.dma_start(lvw[:], tin["lvw_pad16"][:])
            lvb = wp.tile([128, 256], dt.float32, tag="lvb")
            nc.sync.dma_start(lvb[:], tin["lvb_bc"][:])
            qrTs = wp.tile([128, 2, 512], dt.float32, tag="qrTs")
            nc.sync.dma_start(qrTs[:], tin["qrT"][:])
            kpTs = wp.tile([32, 8], dt.float32, tag="kpTs")
            nc.sync.dma_start(kpTs[:], tin["kpT"][:])
            lkws = wp.tile([32, 256], dt.float32, tag="lkws")
            nc.sync.dma_start(lkws[:], tin["lkw"][:])
            lkbc = wp.tile([128, 2], dt.float32, tag="lkbc")
            nc.sync.dma_start(lkbc[:], tin["lkb_col"][:])
            qsel = wp.tile([128, 4], dt.float32, tag="qsel")
            nc.sync.dma_start(qsel[:], tin["qsel_col"][:])

            keyT = wp.tile([128, 2, 8], dt.float32, tag="keyT")
            for cch in range(2):
                kps = ps.tile([128, 8], dt.float32, tag="key")
                nc.tensor.matmul(kps[:], lkws[:, 128 * cch:128 * cch + 128],
                                 kpTs[:], start=True, stop=True)
                nc.scalar.activation(keyT[:, cch, :], kps[:], AF.Sigmoid,
                                     bias=lkbc[:, cch:cch + 1], scale=1.0)

            ag2f = wp.tile([128, 2, 2048], dt.float32, tag="ag2f")
            for cch in range(2):
                nc.sync.dma_start(ag2f[:, cch, :],
                                  agout2[128 * cch:128 * cch + 128, :])
            # select this core's column quarter via the one-hot qsel blend
            ag2sb = wp.tile([128, 2, 512], dt.float32, tag="ag2sb")
            for cch in range(2):
                blendt = wp.tile([128, 512], dt.float32, tag="blendt")
                nc.vector.tensor_scalar(
                    blendt[:], ag2f[:, cch, 0:512], qsel[:, 0:1], None,
                    op0=ALU.mult)
                for qq in range(1, 4):
                    dst = blendt if qq < 3 else None
                    if qq < 3:
                        nc.vector.scalar_tensor_tensor(
                            blendt[:], ag2f[:, cch, 512 * qq:512 * qq + 512],
                            qsel[:, qq:qq + 1], blendt[:],
                            op0=ALU.mult, op1=ALU.add)
                    else:
                        nc.vector.scalar_tensor_tensor(
                            ag2sb[:, cch, :],
                            ag2f[:, cch, 512 * qq:512 * qq + 512],
                            qsel[:, qq:qq + 1], blendt[:],
                            op0=ALU.mult, op1=ALU.add)

            for ic in range(4):
                wops = ps.tile([128, 256], dt.float32, tag="wo3ps")
                for cch in range(2):
                    nc.tensor.matmul(
                        wops[:], ag2sb[:, cch, 128 * ic:128 * ic + 128],
                        wo3[:, cch, :], start=(cch == 0), stop=(cch == 1))
                v = wp.tile([128, 256], dt.float32, tag="v3p")
                nc.vector.tensor_tensor(v[:], wops[:], res3[:], op=ALU.add)
                h3 = wp.tile([128, 256], dt.float32, tag="h3")
                _ln(nc, wp, v, g3, b3, h3[:], "3", epsc)
                h316 = wp.tile([128, 256], dt.float16, tag="h316")
                nc.vector.tensor_copy(h316[:], h3[:])
                h3T = wp.tile([128, 2, 128], dt.float16, tag="h3T")
                nc.sync.dma_start_transpose(h3T[:], h316[:])

                bps = ps.tile([128, 8], dt.float32, tag="beta")
                for cch in range(2):
                    nc.tensor.matmul(
                        bps[:], qrTs[:, cch, 128 * ic:128 * ic + 128],
                        keyT[:, cch, :], start=(cch == 0), stop=(cch == 1))
                bmax = wp.tile([128, 1], dt.float32, tag="bmax")
                nc.vector.tensor_reduce(bmax[:], bps[:], axis=AX.X,
                                        op=ALU.max)
                nbmax = wp.tile([128, 1], dt.float32, tag="nbmax")
                nc.vector.tensor_scalar(nbmax[:], bmax[:], -1.0, None,
                                        op0=ALU.mult)
                ebeta = wp.tile([128, 8], dt.float32, tag="ebeta")
                zb = wp.tile([128, 1], dt.float32, tag="zb")
                nc.scalar.activation(ebeta[:], bps[:], AF.Exp, bias=nbmax[:],
                                     scale=1.0, accum_out=zb[:])
                izb = wp.tile([128, 1], dt.float32, tag="izb")
                nc.vector.reciprocal(izb[:], zb[:])
                alpha = wp.tile([128, 8], dt.float32, tag="alpha")
                nc.vector.tensor_scalar(alpha[:], ebeta[:], izb[:], None,
                                        op0=ALU.mult)

                acc = wp.tile([128, 256], dt.float32, tag="acc")
                accb = wp.tile([128, 256], dt.float32, tag="accb")
                nc.vector.memset(acc[:], 0.0)
                for h in range(8):
                    vps = ps.tile([128, 256], dt.float32, tag="valps")
                    nc.tensor.matmul(vps[:], h3T[:, h // 4, :], lvw[:, h, :],
                                     start=True, stop=True)
                    val = wp.tile([128, 256], dt.float32, tag="val")
                    nc.vector.tensor_tensor(val[:], vps[:], lvb[:],
                                            op=ALU.add)
                    vsg = wp.tile([128, 256], dt.float32, tag="vsg")
                    nc.scalar.activation(vsg[:], val[:], AF.Sigmoid,
                                         bias=0.0, scale=1.0)
                    src, dst2 = (acc, accb) if h % 2 == 0 else (accb, acc)
                    nc.vector.scalar_tensor_tensor(
                        dst2[:], vsg[:], alpha[:, h:h + 1], src[:],
                        op0=ALU.mult, op1=ALU.add)
                nc.sync.dma_start(out_t[128 * ic:128 * ic + 128, :], acc[:])

    nc.finalize()
    return nc


def run(inputs, **kw):
    if "nc" not in _BUILT:
        _BUILT["nc"] = build_bass()
    nc = _BUILT["nc"]
    in_maps = build_in_maps(inputs)
    res = bass_utils.run_bass_kernel_spmd(nc, in_maps,
                                          core_ids=list(range(8)), **kw)
    out = np.zeros((2, 2048, 256), np.float32)
    for c in range(8):
        b, q = c // 4, c % 4
        out[b, 512 * q:512 * q + 512, :] = res.results[c]["out"]
    return out, res


def kernel(**inputs):
    return run(inputs)[0]



# revision 15
# speedup vs baseline: 1.1207x; 1.1207x over previous
"""DTransformer forward on 8 trn2 NeuronCores (bass/Tile, single launch).

Sharding: core c handles batch b=c//4 and head pair p=c%4 (heads 2p, 2p+1)
of ALL three attention blocks. Phase 1 computes blocks 1&2 per-head
attention with the distance-decay bias (4 units/core). A per-batch
AllGather (groups [[0..3],[4..7]]) shares the per-head context in
transposed (feature-major) layout. Phase 2 rebuilds hq/ha (Wo + residual +
layernorm, duplicated inside the batch group) and runs block3 for the
core's 2 heads using the rank-1 structure of block3 scores (the
know_params query row is identical for every position, so the first
softmax reduces to prefix sums of one score vector). A second AllGather
shares block3 context; phase 3 produces a [512, 256] output shard
(row quarter q=c%4 of batch b): Wo3 + layernorm + the gated readout.

Device tricks: suffix tensor_tensor_scan for the decay tail (no
cancellation), ACT Exp/Sqrt with per-partition AP scale/bias, scores kept
fp16, dma_start_transpose (XBAR) for all 128x128 transposes, softmax
denominators via ACT accum_out, fp32r matmuls, ACT table-set grouping
(Exp/Sqrt batched over row-tile groups).
"""
import sys

if "/opt/trn_rl_repo" not in sys.path:
    sys.path.insert(0, "/opt/trn_rl_repo")

import numpy as np

import concourse.bacc as bacc
import concourse.mybir as mybir
import concourse.tile as tile
from concourse import bass_utils

dt = mybir.dt
AF = mybir.ActivationFunctionType
ALU = mybir.AluOpType
AX = mybir.AxisListType

S, D, H, DK, B = 2048, 256, 8, 32, 2
NT = S // 128
ISQ = float(1.0 / np.sqrt(DK))
SQ4 = float(DK ** -0.25)   # dk^(-1/4): folded into Wq so QK^T carries 1/sqrt(dk)
GRP = 16
NEG = -30.0

_BUILT = {}


# --------------------------------------------------------------------------
# host-side input preparation (layout + parameter preprocessing only)
# --------------------------------------------------------------------------

def _softplus(x):
    return np.logaddexp(0.0, x)


def build_in_maps(inp):
    f32, f16 = np.float32, np.float16
    q_emb = np.asarray(inp["q_emb"], f32)
    qa_emb = np.asarray(inp["qa_emb"], f32)

    pos16 = np.maximum(
        np.arange(128)[:, None] + 2048 - np.arange(4096)[None, :], 0
    ).astype(f16)
    i_l = np.arange(128)[:, None]
    j_l = np.arange(128)[None, :]
    tri_pk = (j_l > i_l).astype(np.uint8)
    tri_st = (j_l >= i_l).astype(np.uint8)

    know = np.asarray(inp["know_params"], f32)[0, 0]
    q3 = know @ np.asarray(inp["b3_Wq"], f32) + np.asarray(inp["b3_bq"], f32)
    gam = {k: -_softplus(np.asarray(inp[k + "_gam"], f32)[:, 0, 0])
           for k in ("b1", "b2", "b3")}
    Wq = [np.asarray(inp["b1_Wq"], f32), np.asarray(inp["b2_Wq"], f32)]
    Wv = [np.asarray(inp["b1_Wv"], f32), np.asarray(inp["b2_Wv"], f32)]
    bq = [np.asarray(inp["b1_bq"], f32), np.asarray(inp["b2_bq"], f32)]
    bv = [np.asarray(inp["b1_bv"], f32), np.asarray(inp["b2_bv"], f32)]
    Wk3 = np.asarray(inp["b3_Wk"], f32)
    bk3 = np.asarray(inp["b3_bk"], f32)
    lvW = np.asarray(inp["lv_W"], f32)

    def chunk2(w):   # [256, F] -> [128, 2, F]
        return np.ascontiguousarray(w.reshape(2, 128, -1).transpose(1, 0, 2))

    def bc(v):       # [256] -> [128, 256] broadcast
        return np.broadcast_to(np.asarray(v, f32), (128, 256)).copy()

    lvw_pad = np.zeros((128, 8, 256), f16)
    for h in range(8):
        r0 = 32 * (h % 4)
        lvw_pad[r0:r0 + 32, h, :] = lvW.astype(f16)

    maps = []
    for c in range(8):
        b = c // 4
        p = c % 4
        heads = [2 * p, 2 * p + 1]
        X = [q_emb[b], qa_emb[b]]            # [2][2048, 256]

        xT = np.zeros((128, 2, 2, 2048), f32)
        wq_s = np.zeros((128, 2, 2, 2, 32), f32)
        wv_s = np.zeros((128, 2, 2, 64), f32)
        bq_col = np.zeros((32, 2, 2), f32)
        bv_bc = np.zeros((128, 2, 64), f32)
        gam_col = np.zeros((128, 2, 2), f32)
        for blk in range(2):
            xT[:, :, blk, :] = X[blk].T.reshape(2, 128, 2048).transpose(1, 0, 2)
            for hi, h in enumerate(heads):
                hs = slice(32 * h, 32 * h + 32)
                wq_s[:, :, blk, hi, :] = chunk2(Wq[blk][:, hs]) * SQ4
                bq_col[:, blk, hi] = bq[blk][hs] * SQ4
                gam_col[:, blk, hi] = gam[("b1", "b2")[blk]][h]
            wv_s[:, :, blk, :] = chunk2(Wv[blk][:, 64 * p:64 * p + 64])
            bv_bc[:, blk, :] = bv[blk][64 * p:64 * p + 64]

        w16 = np.zeros((128, 2, 2), f16)
        ch_col = np.zeros((1, 2), f32)
        for hi, h in enumerate(heads):
            hs = slice(32 * h, 32 * h + 32)
            w = Wk3[:, hs] @ q3[hs]
            w16[:, :, hi] = w.reshape(2, 128).T.astype(f16)
            ch_col[0, hi] = float((bk3[hs] * q3[hs]).sum() * ISQ)

        qsel = np.zeros((128, 4), f32)
        qsel[:, p] = 1.0

        m = {
            "xT": xT,
            "wq_s": wq_s,
            "bq_col": bq_col,
            "wv_s": wv_s,
            "bv_bc": bv_bc,
            "gam_col": gam_col,
            "pos16": pos16,
            "tri_pk": tri_pk,
            "tri_st": tri_st,
            "r1": q_emb[b] + np.asarray(inp["b1_bo"], f32),
            "r2": qa_emb[b] + np.asarray(inp["b2_bo"], f32),
            "wo1": chunk2(np.asarray(inp["b1_Wo"], f32)),
            "wo2": chunk2(np.asarray(inp["b2_Wo"], f32)),
            "g1_bc": bc(inp["b1_lng"]), "b1_bc": bc(inp["b1_lnb"]),
            "g2_bc": bc(inp["b2_lng"]), "b2_bc": bc(inp["b2_lnb"]),
            "w16": w16,
            "ch_col": ch_col,
            "wv3_s16": chunk2(np.asarray(inp["b3_Wv"], f32)
                              [:, 64 * p:64 * p + 64]).astype(f16),
            "bv3_bc": np.broadcast_to(
                np.asarray(inp["b3_bv"], f32)[64 * p:64 * p + 64],
                (128, 64)).copy(),
            "gam3_col": np.broadcast_to(gam["b3"][heads], (128, 2)).copy(),
            # block3 ctx is gathered per head-pair half (heads 0,2,4,6 then
            # 1,3,5,7) -> permute Wo3 input rows to match
            "wo3": chunk2(np.asarray(inp["b3_Wo"], f32)[
                [32 * h + k for h in (0, 2, 4, 6, 1, 3, 5, 7)
                 for k in range(32)], :]),
            "res3_bc": bc(know + np.asarray(inp["b3_bo"], f32)),
            "g3_bc": bc(inp["b3_lng"]), "b3_bc": bc(inp["b3_lnb"]),
            "lvw_pad16": lvw_pad,
            "lvb_bc": bc(inp["lv_b"]),
            "qrT": np.ascontiguousarray(
                q_emb[b, 512 * p:512 * p + 512].T
                .reshape(2, 128, 512).transpose(1, 0, 2)),
            "kpT": np.ascontiguousarray(know.reshape(8, 32).T),
            "lkw": np.asarray(inp["lk_W"], f32),
            "lkb_col": np.ascontiguousarray(
                np.asarray(inp["lk_b"], f32).reshape(2, 128).T),
            "qsel_col": qsel,
        }
        maps.append(m)
    return maps


# --------------------------------------------------------------------------
# bass program
# --------------------------------------------------------------------------

def _ln(nc, pool, v, g_sb, b_sb, out, tag, eps):
    """out = layernorm(v) * g + b, v fp32 [128, 256]."""
    sv = pool.tile([128, 1], dt.float32, tag=f"sv{tag}")
    nc.vector.tensor_reduce(sv[:], v[:], axis=AX.X, op=ALU.add)
    sq = pool.tile([128, 256], dt.float32, tag=f"sq{tag}")
    s2v = pool.tile([128, 1], dt.float32, tag=f"s2v{tag}")
    nc.scalar.activation(sq[:], v[:], AF.Square, bias=0.0, scale=1.0,
                         accum_out=s2v[:])
    mu = pool.tile([128, 1], dt.float32, tag=f"mu{tag}")
    nc.vector.tensor_scalar(mu[:], sv[:], 1.0 / 256, None, op0=ALU.mult)
    mu2 = pool.tile([128, 1], dt.float32, tag=f"mu2{tag}")
    nc.vector.tensor_tensor(mu2[:], mu[:], mu[:], op=ALU.mult)
    var = pool.tile([128, 1], dt.float32, tag=f"var{tag}")
    nc.vector.scalar_tensor_tensor(var[:], s2v[:], 1.0 / 256, mu2[:],
                                   op0=ALU.mult, op1=ALU.subtract)
    sd = pool.tile([128, 1], dt.float32, tag=f"sd{tag}")
    nc.scalar.activation(sd[:], var[:], AF.Sqrt, bias=eps[:], scale=1.0)
    rstd = pool.tile([128, 1], dt.float32, tag=f"rstd{tag}")
    nc.vector.reciprocal(rstd[:], sd[:])
    xn = pool.tile([128, 256], dt.float32, tag=f"xn{tag}")
    nc.vector.tensor_scalar(xn[:], v[:], mu[:], rstd[:],
                            op0=ALU.subtract, op1=ALU.mult)
    nc.vector.tensor_tensor(xn[:], xn[:], g_sb[:], op=ALU.mult)
    nc.vector.tensor_tensor(out, xn[:], b_sb[:], op=ALU.add)


def _attn_triangle(nc, wp, ctxps, sc_mm, gam_ap, Vtile, v_off,
                   pos, tri, neg16, agdst, agrow, strict, sbc=None,
                   Pbc=None, ninvP=None, ones16=None, grp=GRP):
    """The per-unit decay-bias attention triangle (16 row tiles, grouped).

    Blocks 1/2 path: sc_mm(t, f, fw, tag) emits the scores matmul chunk
    into PSUM and returns the psum tile (stage A reads it through Exp,
    stage D re-issues it for the s2 product — cheaper than keeping an
    fp16 score copy in SBUF). Block3 path (strict=True, sbc/Pbc given):
    rank-1 scores, no matmul/scan. tail/t1 are sums/ratios of
    exponentials so the reference's max(.,0) clamps are dead — plain
    tensor_tensor mult gets the 2x fp16 DVE mode.
    """
    for g0 in range(0, NT, grp):
        tiles = list(range(g0, min(g0 + grp, NT)))
        chain = {}
        if not strict:
            # stage A: scores -> e (Exp); 1/sqrt(dk) is folded into Wq
            for t in tiles:
                W = 128 * (t + 1)
                e = wp.tile([128, 2048], dt.float16, tag=f"ch{t - g0}", name="e")
                chain[t] = e
                for f in range((W + 511) // 512):
                    fw = min(512, W - 512 * f)
                    ps = sc_mm(t, f, fw, "sc")
                    nc.scalar.activation(e[:, 512 * f:512 * f + fw],
                                         ps[:, :fw], AF.Exp, bias=0.0,
                                         scale=1.0)
                nc.vector.copy_predicated(e[:, W - 128:W], tri[:], neg16[1])
            # stage B: suffix scan -> tail, u = tail*pos
            invZ = wp.tile([128, grp], dt.float32, tag="invZ")
            usb = {}
            for t in tiles:
                W = 128 * (t + 1)
                suf = wp.tile([128, 2049], dt.float16, tag=f"sf{t - g0}", name="suf")
                nc.vector.memset(suf[:, W:W + 1], 0.0)
                nc.vector.tensor_tensor_scan(
                    suf[:, :W][:, ::-1], chain[t][:, :W][:, ::-1],
                    chain[t][:, :W][:, ::-1], 0.0, op0=ALU.add, op1=ALU.bypass)
                nc.vector.reciprocal(invZ[:, t - g0:t - g0 + 1], suf[:, 0:1])
                u = wp.tile([128, 2048], dt.float16, tag=f"ch{t - g0}", name="u")
                nc.vector.tensor_tensor(
                    u[:, :W], suf[:, 1:W + 1],
                    pos[:, 2048 - 128 * t:2048 - 128 * t + W], op=ALU.mult)
                usb[t] = u
        else:
            # block3 stage B': t1 = P*(-1/Pprev) + 1, u = t1*pos
            usb = {}
            invZ = None
            for t in tiles:
                W = 128 * (t + 1)
                t1 = wp.tile([128, 2048], dt.float16, tag=f"sf{t - g0}", name="t1")
                nc.vector.scalar_tensor_tensor(
                    t1[:, :W], Pbc[:, :W], ninvP[:, t:t + 1], ones16[:, :W],
                    op0=ALU.mult, op1=ALU.add)
                u = wp.tile([128, 2048], dt.float16, tag=f"ch{t - g0}", name="u")
                nc.vector.tensor_tensor(
                    u[:, :W], t1[:, :W],
                    pos[:, 2048 - 128 * t:2048 - 128 * t + W], op=ALU.mult)
                usb[t] = u
        # stage C: r = sqrt(u * invZ)  (invZ=1 for block3)
        rsb = {}
        for t in tiles:
            W = 128 * (t + 1)
            r = wp.tile([128, 2048], dt.float16, tag=f"sf{t - g0}", name="r")
            if strict:
                nc.scalar.activation(r[:, :W], usb[t][:, :W], AF.Sqrt,
                                     bias=0.0, scale=1.0)
            else:
                nc.scalar.activation(r[:, :W], usb[t][:, :W], AF.Sqrt,
                                     bias=0.0, scale=invZ[:, t - g0:t - g0 + 1])
            rsb[t] = r
        # stage D: eff, scores2, e2, p2, transpose, ctx matmul, store
        for t in tiles:
            W = 128 * (t + 1)
            m0 = wp.tile([128, 2048], dt.float16, tag=f"ch{t - g0}", name="m0")
            nc.scalar.activation(m0[:, :W], rsb[t][:, :W], AF.Exp,
                                 bias=0.0, scale=gam_ap)
            s2 = wp.tile([128, 2048], dt.float16, tag=f"sf{t - g0}", name="s2")
            if strict:
                nc.vector.tensor_tensor(s2[:, :W], m0[:, :W], sbc[:, :W],
                                        op=ALU.mult)
            else:
                for f in range((W + 511) // 512):
                    fw = min(512, W - 512 * f)
                    ps = sc_mm(t, f, fw, "sc2")
                    nc.vector.tensor_tensor(
                        s2[:, 512 * f:512 * f + fw],
                        m0[:, 512 * f:512 * f + fw], ps[:, :fw], op=ALU.mult)
            nc.vector.copy_predicated(s2[:, W - 128:W], tri[:], neg16[0])
            e2 = wp.tile([128, 2048], dt.float16, tag=f"ch{t - g0}", name="e2")
            Z2 = wp.tile([128, 1], dt.float32, tag="Z2")
            nc.scalar.activation(e2[:, :W], s2[:, :W], AF.Exp, bias=0.0,
                                 scale=1.0, accum_out=Z2[:])
            iZ2 = wp.tile([128, 1], dt.float32, tag="iZ2")
            nc.vector.reciprocal(iZ2[:], Z2[:])
            p2 = wp.tile([128, 2048], dt.float16, tag=f"sf{t - g0}", name="p2")
            nc.vector.tensor_scalar(p2[:, :W], e2[:, :W], iZ2[:], None,
                                    op0=ALU.mult)
            if strict and t == 0:
                nc.vector.memset(p2[0:1, 0:128], 0.0)
            p2T = wp.tile([128, 16, 128], dt.float16, tag=f"ch{t - g0}", name="p2T")
            nc.sync.dma_start_transpose(p2T[:, :t + 1, :], p2[:, :W])
            cps = ctxps.tile([32, 128], dt.float32, tag="ctx")
            for jb in range(t + 1):
                nc.tensor.matmul(cps[:], Vtile[:, jb, v_off:v_off + 32],
                                 p2T[:, jb, :], start=(jb == 0), stop=(jb == t))
            csb = wp.tile([32, 128], dt.float32, tag="csb", name="csb")
            nc.vector.tensor_copy(csb[:], cps[:])
            nc.sync.dma_start(
                agdst[agrow:agrow + 32, 128 * t:128 * t + 128], csb[:])


def build_bass(sim=False):
    nc = bacc.Bacc("TRN2", target_bir_lowering=False)

    def din(name, shape, dtyp=dt.float32):
        return nc.dram_tensor(name, shape, dtyp, kind="ExternalInput")

    tin = {
        "xT": din("xT", [128, 2, 2, 2048], dt.float32r),
        "wq_s": din("wq_s", [128, 2, 2, 2, 32], dt.float32r),
        "bq_col": din("bq_col", [32, 2, 2]),
        "wv_s": din("wv_s", [128, 2, 2, 64], dt.float32r),
        "bv_bc": din("bv_bc", [128, 2, 64]),
        "gam_col": din("gam_col", [128, 2, 2]),
        "pos16": din("pos16", [128, 4096], dt.float16),
        "tri_pk": din("tri_pk", [128, 128], dt.uint8),
        "tri_st": din("tri_st", [128, 128], dt.uint8),
        "r1": din("r1", [2048, 256]),
        "r2": din("r2", [2048, 256]),
        "wo1": din("wo1", [128, 2, 256], dt.float32r),
        "wo2": din("wo2", [128, 2, 256], dt.float32r),
        "g1_bc": din("g1_bc", [128, 256]), "b1_bc": din("b1_bc", [128, 256]),
        "g2_bc": din("g2_bc", [128, 256]), "b2_bc": din("b2_bc", [128, 256]),
        "w16": din("w16", [128, 2, 2], dt.float16),
        "ch_col": din("ch_col", [1, 2]),
        "wv3_s16": din("wv3_s16", [128, 2, 64], dt.float16),
        "bv3_bc": din("bv3_bc", [128, 64]),
        "gam3_col": din("gam3_col", [128, 2]),
        "wo3": din("wo3", [128, 2, 256]),
        "res3_bc": din("res3_bc", [128, 256]),
        "g3_bc": din("g3_bc", [128, 256]), "b3_bc": din("b3_bc", [128, 256]),
        "lvw_pad16": din("lvw_pad16", [128, 8, 256], dt.float16),
        "lvb_bc": din("lvb_bc", [128, 256]),
        "qrT": din("qrT", [128, 2, 512]),
        "kpT": din("kpT", [32, 8]),
        "lkw": din("lkw", [32, 256]),
        "lkb_col": din("lkb_col", [128, 2]),
        "qsel_col": din("qsel_col", [128, 4]),
    }
    out_t = nc.dram_tensor("out", [512, 256], dt.float32,
                           kind="ExternalOutput")

    with tile.TileContext(nc) as tc, \
         tc.tile_pool(name="consts", bufs=1) as cs, \
         tc.tile_pool(name="dram", bufs=1, space="DRAM") as dram:
        pos = cs.tile([128, 4096], dt.float16)
        nc.sync.dma_start(pos[:], tin["pos16"][:])
        tpk = cs.tile([128, 128], dt.uint8)
        nc.sync.dma_start(tpk[:], tin["tri_pk"][:])
        tst = cs.tile([128, 128], dt.uint8)
        nc.sync.dma_start(tst[:], tin["tri_st"][:])
        gamc = cs.tile([128, 2, 2], dt.float32)
        nc.sync.dma_start(gamc[:], tin["gam_col"][:])
        gam3c = cs.tile([128, 2], dt.float32)
        nc.sync.dma_start(gam3c[:], tin["gam3_col"][:])
        bqc = cs.tile([32, 2, 2], dt.float32)
        nc.sync.dma_start(bqc[:], tin["bq_col"][:])
        bvb = cs.tile([128, 2, 64], dt.float32)
        nc.sync.dma_start(bvb[:], tin["bv_bc"][:])
        neg16 = cs.tile([128, 128], dt.float16)
        nc.vector.memset(neg16[:], NEG)
        zero16 = cs.tile([128, 128], dt.float16)
        nc.vector.memset(zero16[:], 0.0)
        ones16 = cs.tile([128, 2048], dt.float16)
        nc.vector.memset(ones16[:], 1.0)
        tch = cs.tile([1, 4], dt.float16)
        nc.vector.tensor_copy(tch[:, 0:1], pos[:1, :1])
        epsc = cs.tile([128, 1], dt.float32)
        nc.vector.memset(epsc[:], 1e-5)

        # per-block / per-head-pair gather halves so each collective can
        # overlap the next unit's compute instead of stalling the core
        agin1a = dram.tile([64, 2048], dt.float32)
        agout1a = dram.tile([256, 2048], dt.float32)
        agin1b = dram.tile([64, 2048], dt.float32)
        agout1b = dram.tile([256, 2048], dt.float32)
        agin2a = dram.tile([32, 2048], dt.float32)
        agout2a = dram.tile([128, 2048], dt.float32)
        agin2b = dram.tile([32, 2048], dt.float32)
        agout2b = dram.tile([128, 2048], dt.float32)
        pbuf = dram.tile([2, 2049], dt.float32)

        # ---------------- phase 1 ----------------
        qtv_cm = tc.tile_pool(name="qtv", bufs=1)
        qtv = qtv_cm.__enter__()
        QT = qtv.tile([32, 2, 2, 2048], dt.float16)
        V16 = qtv.tile([128, 2, 16, 64], dt.float16)
        with tc.tile_pool(name="proj", bufs=1) as pj, \
             tc.tile_pool(name="pjps", bufs=2, space="PSUM") as pjps:
            xTs = pj.tile([128, 2, 2, 2048], dt.float32r)
            nc.sync.dma_start(xTs[:], tin["xT"][:])
            wqs = pj.tile([128, 2, 2, 2, 32], dt.float32r)
            nc.sync.dma_start(wqs[:], tin["wq_s"][:])
            wvs = pj.tile([128, 2, 2, 64], dt.float32r)
            nc.sync.dma_start(wvs[:], tin["wv_s"][:])
            for blk in range(2):
                for hi in range(2):
                    for f in range(4):
                        ps = pjps.tile([32, 512], dt.float32, tag="qt")
                        for cch in range(2):
                            nc.tensor.matmul(
                                ps[:], wqs[:, cch, blk, hi, :],
                                xTs[:, cch, blk, 512 * f:512 * f + 512],
                                start=(cch == 0), stop=(cch == 1))
                        nc.scalar.activation(
                            QT[:, blk, hi, 512 * f:512 * f + 512], ps[:],
                            AF.Identity, bias=bqc[:, blk, hi:hi + 1],
                            scale=1.0)
                for jb in range(16):
                    ps = pjps.tile([128, 64], dt.float32, tag="v")
                    for cch in range(2):
                        nc.tensor.matmul(
                            ps[:], xTs[:, cch, blk, 128 * jb:128 * jb + 128],
                            wvs[:, cch, blk, :], start=(cch == 0),
                            stop=(cch == 1))
                    nc.vector.tensor_tensor(V16[:, blk, jb, :], ps[:],
                                            bvb[:, blk, :], op=ALU.add)

        with tc.tile_pool(name="p1", bufs=1) as wp, \
             tc.tile_pool(name="scps", bufs=3, space="PSUM") as scps, \
             tc.tile_pool(name="ctxps", bufs=2, space="PSUM") as ctxps:
            for blk in range(2):
                agdst = (agin1a, agin1b)[blk]
                for hi in range(2):
                    def sc_mm(t, f, fw, tag, blk=blk, hi=hi):
                        ps = scps.tile([128, 512], dt.float32, tag=tag)
                        nc.tensor.matmul(
                            ps[:, :fw],
                            QT[:, blk, hi, 128 * t:128 * t + 128],
                            QT[:, blk, hi, 512 * f:512 * f + fw],
                            start=True, stop=True)
                        return ps
                    _attn_triangle(
                        nc, wp, ctxps, sc_mm,
                        gamc[:, blk, hi:hi + 1], V16[:, blk], 32 * hi,
                        pos, tpk, (neg16, zero16), agdst,
                        32 * hi, strict=False)
                # gather this block's context now; overlaps the next
                # block's triangles / the hq rebuild
                agout = (agout1a, agout1b)[blk]
                if sim:
                    for rr in range(4):
                        nc.gpsimd.dma_start(agout[64 * rr:64 * rr + 64, :],
                                            agdst[:])
                else:
                    nc.gpsimd.collective_compute(
                        "AllGather", ALU.bypass, ins=[agdst.opt()],
                        outs=[agout.opt()],
                        replica_groups=[[0, 1, 2, 3], [4, 5, 6, 7]])

        qtv_cm.__exit__(None, None, None)

        # ---------------- phase 2: hq/ha ----------------
        hq16 = cs.tile([128, 2, 2048], dt.float16)
        ha16 = cs.tile([128, 2, 2048], dt.float16)
        with tc.tile_pool(name="p2", bufs=1) as p2, \
             tc.tile_pool(name="p2ps", bufs=2, space="PSUM") as p2ps:
            for which, (wo_n, res_n, g_n, bb_n, dstT, agsrc) in enumerate([
                    ("wo1", "r1", "g1_bc", "b1_bc", hq16, agout1a),
                    ("wo2", "r2", "g2_bc", "b2_bc", ha16, agout1b)]):
                wo_sb = p2.tile([128, 2, 256], dt.float32r, tag="wo")
                nc.sync.dma_start(wo_sb[:], tin[wo_n][:])
                g_sb = p2.tile([128, 256], dt.float32, tag="g")
                nc.sync.dma_start(g_sb[:], tin[g_n][:])
                bb_sb = p2.tile([128, 256], dt.float32, tag="bb")
                nc.sync.dma_start(bb_sb[:], tin[bb_n][:])
                ctxT = p2.tile([128, 2, 2048], dt.float32r, tag="ctxT")
                for cch in range(2):
                    nc.gpsimd.dma_start(
                        ctxT[:, cch, :],
                        agsrc[128 * cch:128 * cch + 128, :])
                h16 = p2.tile([128, 16, 256], dt.float16, tag="h16")
                for ic in range(16):
                    ps = p2ps.tile([128, 256], dt.float32, tag="wops")
                    for cch in range(2):
                        nc.tensor.matmul(
                            ps[:], ctxT[:, cch, 128 * ic:128 * ic + 128],
                            wo_sb[:, cch, :], start=(cch == 0),
                            stop=(cch == 1))
                    res_sb = p2.tile([128, 256], dt.float32, tag="res")
                    nc.sync.dma_start(res_sb[:],
                                      tin[res_n][128 * ic:128 * ic + 128, :])
                    v = p2.tile([128, 256], dt.float32, tag="v")
                    nc.vector.tensor_tensor(v[:], ps[:], res_sb[:], op=ALU.add)
                    _ln(nc, p2, v, g_sb, bb_sb, h16[:, ic, :], "a", epsc)
                for ic in range(16):
                    nc.sync.dma_start_transpose(
                        dstT[:, :, 128 * ic:128 * ic + 128], h16[:, ic, :])

        # ---------------- phase 2: block3 ----------------
        with tc.tile_pool(name="b3", bufs=1) as wp, \
             tc.tile_pool(name="b3ps", bufs=2, space="PSUM") as ps3, \
             tc.tile_pool(name="b3ctx", bufs=2, space="PSUM") as ctxps:
            w16sb = wp.tile([128, 2, 2], dt.float16, tag="w16")
            nc.sync.dma_start(w16sb[:], tin["w16"][:])
            chc = wp.tile([1, 2], dt.float32, tag="chc")
            nc.sync.dma_start(chc[:], tin["ch_col"][:])
            wv3 = wp.tile([128, 2, 64], dt.float16, tag="wv3")
            nc.sync.dma_start(wv3[:], tin["wv3_s16"][:])
            bv3 = wp.tile([128, 64], dt.float32, tag="bv3")
            nc.sync.dma_start(bv3[:], tin["bv3_bc"][:])
            one1 = wp.tile([1, 1], dt.float32, tag="one1")
            nc.vector.memset(one1[:], 1.0)
            onesrow = wp.tile([1, 128], dt.float32, tag="onesrow")
            nc.vector.memset(onesrow[:], 1.0)

            V3 = wp.tile([128, 16, 64], dt.float16, tag="V3")
            for jb in range(16):
                vps = ps3.tile([128, 64], dt.float32, tag="v3")
                for cch in range(2):
                    nc.tensor.matmul(
                        vps[:], ha16[:, cch, 128 * jb:128 * jb + 128],
                        wv3[:, cch, :], start=(cch == 0), stop=(cch == 1))
                nc.vector.tensor_tensor(V3[:, jb, :], vps[:], bv3[:],
                                        op=ALU.add)

            for hi in range(2):
                s_row = wp.tile([1, 2048], dt.float32, tag="srow")
                for f in range(4):
                    sp = ps3.tile([1, 512], dt.float32, tag="s")
                    for cch in range(2):
                        nc.tensor.matmul(
                            sp[:], w16sb[:, cch, hi:hi + 1],
                            hq16[:, cch, 512 * f:512 * f + 512],
                            start=(cch == 0), stop=(cch == 1))
                    nc.scalar.activation(
                        s_row[:, 512 * f:512 * f + 512], sp[:], AF.Identity,
                        bias=chc[:, hi:hi + 1], scale=ISQ)
                smax = wp.tile([1, 1], dt.float32, tag="smax")
                nc.vector.tensor_reduce(smax[:], s_row[:], axis=AX.X,
                                        op=ALU.max)
                nsmax = wp.tile([1, 1], dt.float32, tag="nsmax")
                nc.vector.tensor_scalar(nsmax[:], smax[:], -1.0, None,
                                        op0=ALU.mult)
                e3 = wp.tile([1, 2048], dt.float32, tag="e3")
                nc.scalar.activation(e3[:], s_row[:], AF.Exp, bias=nsmax[:],
                                     scale=1.0)
                P_row = wp.tile([1, 2048], dt.float32, tag="Prow")
                nc.vector.tensor_tensor_scan(P_row[:], e3[:], e3[:], 0.0,
                                             op0=ALU.add, op1=ALU.bypass)
                nc.sync.dma_start(pbuf[hi, 0:1], one1[:])
                nc.sync.dma_start(pbuf[hi, 1:2049], P_row[:])
                npcol = wp.tile([128, 16], dt.float32, tag="npcol")
                pcol = wp.tile([128, 16], dt.float32, tag="pcol")
                nc.sync.dma_start(
                    pcol[:], pbuf[hi, 0:2048].rearrange("(t p) -> p t", p=128))
                nc.vector.tensor_scalar(pcol[:], pcol[:], -1.0, None,
                                        op0=ALU.mult)
                nc.vector.reciprocal(npcol[:], pcol[:])
                P_bc = wp.tile([128, 2048], dt.float32, tag="Pbc")
                s_bc = wp.tile([128, 2048], dt.float16, tag="sbc")
                for f in range(4):
                    bp = ps3.tile([128, 512], dt.float32, tag="bc")
                    nc.tensor.matmul(bp[:], onesrow[:],
                                     P_row[:, 512 * f:512 * f + 512],
                                     start=True, stop=True)
                    nc.vector.tensor_copy(P_bc[:, 512 * f:512 * f + 512],
                                          bp[:])
                    bs = ps3.tile([128, 512], dt.float32, tag="bc")
                    nc.tensor.matmul(bs[:], onesrow[:],
                                     s_row[:, 512 * f:512 * f + 512],
                                     start=True, stop=True)
                    nc.vector.tensor_copy(s_bc[:, 512 * f:512 * f + 512],
                                          bs[:])
                agdst = (agin2a, agin2b)[hi]
                _attn_triangle(
                    nc, wp, ctxps, None, gam3c[:, hi:hi + 1],
                    V3, 32 * hi, pos, tst, (neg16, zero16), agdst, 0,
                    strict=True, sbc=s_bc, Pbc=P_bc, ninvP=npcol,
                    ones16=ones16)
                agout = (agout2a, agout2b)[hi]
                if sim:
                    for rr in range(4):
                        nc.gpsimd.dma_start(
                            agout[32 * rr:32 * rr + 32, :], agdst[:])
                else:
                    nc.gpsimd.collective_compute(
                        "AllGather", ALU.bypass, ins=[agdst.opt()],
                        outs=[agout.opt()],
                        replica_groups=[[0, 1, 2, 3], [4, 5, 6, 7]])

        # ---------------- phase 3 ----------------
        with tc.tile_pool(name="p3", bufs=1) as wp, \
             tc.tile_pool(name="p3ps", bufs=2, space="PSUM") as ps:
            wo3 = wp.tile([128, 2, 256], dt.float32, tag="wo3")
            nc.sync.dma_start(wo3[:], tin["wo3"][:])
            res3 = wp.tile([128, 256], dt.float32, tag="res3")
            nc.sync.dma_start(res3[:], tin["res3_bc"][:])
            g3 = wp.tile([128, 256], dt.float32, tag="g3")
            nc.sync.dma_start(g3[:], tin["g3_bc"][:])
            b3 = wp.tile([128, 256], dt.float32, tag="b3")
            nc.sync.dma_start(b3[:], tin["b3_bc"][:])
            lvw = wp.tile([128, 8, 256], dt.float16, tag="lvw")
            nc.sync.dma_start(lvw[:], tin["lvw_pad16"][:])
            lvb = wp.tile([128, 256], dt.float32, tag="lvb")
            nc.sync.dma_start(lvb[:], tin["lvb_bc"][:])
            qrTs = wp.tile([128, 2, 512], dt.float32, tag="qrTs")
            nc.sync.dma_start(qrTs[:], tin["qrT"][:])
            kpTs = wp.tile([32, 8], dt.float32, tag="kpTs")
            nc.sync.dma_start(kpTs[:], tin["kpT"][:])
            lkws = wp.tile([32, 256], dt.float32, tag="lkws")
            nc.sync.dma_start(lkws[:], tin["lkw"][:])
            lkbc = wp.tile([128, 2], dt.float32, tag="lkbc")
            nc.sync.dma_start(lkbc[:], tin["lkb_col"][:])
            qsel = wp.tile([128, 4], dt.float32, tag="qsel")
            nc.sync.dma_start(qsel[:], tin["qsel_col"][:])

            keyT = wp.tile([128, 2, 8], dt.float32, tag="keyT")
            for cch in range(2):
                kps = ps.tile([128, 8], dt.float32, tag="key")
                nc.tensor.matmul(kps[:], lkws[:, 128 * cch:128 * cch + 128],
                                 kpTs[:], start=True, stop=True)
                nc.scalar.activation(keyT[:, cch, :], kps[:], AF.Sigmoid,
                                     bias=lkbc[:, cch:cch + 1], scale=1.0)

            ag2f = wp.tile([128, 2, 2048], dt.float32, tag="ag2f")
            for cch, agsrc in enumerate((agout2a, agout2b)):
                nc.sync.dma_start(ag2f[:, cch, :], agsrc[:])
            # select this core's column quarter via the one-hot qsel blend
            ag2sb = wp.tile([128, 2, 512], dt.float32, tag="ag2sb")
            for cch in range(2):
                blendt = wp.tile([128, 512], dt.float32, tag="blendt")
                nc.vector.tensor_scalar(
                    blendt[:], ag2f[:, cch, 0:512], qsel[:, 0:1], None,
                    op0=ALU.mult)
                for qq in range(1, 4):
                    dst = blendt if qq < 3 else None
                    if qq < 3:
                        nc.vector.scalar_tensor_tensor(
                            blendt[:], ag2f[:, cch, 512 * qq:512 * qq + 512],
                            qsel[:, qq:qq + 1], blendt[:],
                            op0=ALU.mult, op1=ALU.add)
                    else:
                        nc.vector.scalar_tensor_tensor(
                            ag2sb[:, cch, :],
                            ag2f[:, cch, 512 * qq:512 * qq + 512],
                            qsel[:, qq:qq + 1], blendt[:],
                            op0=ALU.mult, op1=ALU.add)

            for ic in range(4):
                wops = ps.tile([128, 256], dt.float32, tag="wo3ps")
                for cch in range(2):
                    nc.tensor.matmul(
                        wops[:], ag2sb[:, cch, 128 * ic:128 * ic + 128],
                        wo3[:, cch, :], start=(cch == 0), stop=(cch == 1))
                v = wp.tile([128, 256], dt.float32, tag="v3p")
                nc.vector.tensor_tensor(v[:], wops[:], res3[:], op=ALU.add)
                h3 = wp.tile([128, 256], dt.float32, tag="h3")
                _ln(nc, wp, v, g3, b3, h3[:], "3", epsc)
                h316 = wp.tile([128, 256], dt.float16, tag="h316")
                nc.vector.tensor_copy(h316[:], h3[:])
                h3T = wp.tile([128, 2, 128], dt.float16, tag="h3T")
                nc.sync.dma_start_transpose(h3T[:], h316[:])

                bps = ps.tile([128, 8], dt.float32, tag="beta")
                for cch in range(2):
                    nc.tensor.matmul(
                        bps[:], qrTs[:, cch, 128 * ic:128 * ic + 128],
                        keyT[:, cch, :], start=(cch == 0), stop=(cch == 1))
                bmax = wp.tile([128, 1], dt.float32, tag="bmax")
                nc.vector.tensor_reduce(bmax[:], bps[:], axis=AX.X,
                                        op=ALU.max)
                nbmax = wp.tile([128, 1], dt.float32, tag="nbmax")
                nc.vector.tensor_scalar(nbmax[:], bmax[:], -1.0, None,
                                        op0=ALU.mult)
                ebeta = wp.tile([128, 8], dt.float32, tag="ebeta")
                zb = wp.tile([128, 1], dt.float32, tag="zb")
                nc.scalar.activation(ebeta[:], bps[:], AF.Exp, bias=nbmax[:],
                                     scale=1.0, accum_out=zb[:])
                izb = wp.tile([128, 1], dt.float32, tag="izb")
                nc.vector.reciprocal(izb[:], zb[:])
                alpha = wp.tile([128, 8], dt.float32, tag="alpha")
                nc.vector.tensor_scalar(alpha[:], ebeta[:], izb[:], None,
                                        op0=ALU.mult)

                acc = wp.tile([128, 256], dt.float32, tag="acc")
                accb = wp.tile([128, 256], dt.float32, tag="accb")
                nc.vector.memset(acc[:], 0.0)
                for h in range(8):
                    vps = ps.tile([128, 256], dt.float32, tag="valps")
                    nc.tensor.matmul(vps[:], h3T[:, h // 4, :], lvw[:, h, :],
                                     start=True, stop=True)
                    val = wp.tile([128, 256], dt.float32, tag="val")
                    nc.vector.tensor_tensor(val[:], vps[:], lvb[:],
                                            op=ALU.add)
                    vsg = wp.tile([128, 256], dt.float32, tag="vsg")
                    nc.scalar.activation(vsg[:], val[:], AF.Sigmoid,
                                         bias=0.0, scale=1.0)
                    src, dst2 = (acc, accb) if h % 2 == 0 else (accb, acc)
                    nc.vector.scalar_tensor_tensor(
                        dst2[:], vsg[:], alpha[:, h:h + 1], src[:],
                        op0=ALU.mult, op1=ALU.add)
                nc.sync.dma_start(out_t[128 * ic:128 * ic + 128, :], acc[:])

    nc.finalize()
    return nc


def run(inputs, **kw):
    if "nc" not in _BUILT:
        _BUILT["nc"] = build_bass()
    nc = _BUILT["nc"]
    in_maps = build_in_maps(inputs)
    res = bass_utils.run_bass_kernel_spmd(nc, in_maps,
                                          core_ids=list(range(8)), **kw)
    out = np.zeros((2, 2048, 256), np.float32)
    for c in range(8):
        b, q = c // 4, c % 4
        out[b, 512 * q:512 * q + 512, :] = res.results[c]["out"]
    return out, res


def kernel(**inputs):
    return run(inputs)[0]



# revision 20
# speedup vs baseline: 1.1894x; 1.0613x over previous
"""DTransformer forward on 8 trn2 NeuronCores (bass/Tile, single launch).

Sharding: core c handles batch b=c//4 and head pair p=c%4 (heads 2p, 2p+1)
of ALL three attention blocks. Phase 1 computes blocks 1&2 per-head
attention with the distance-decay bias (4 units/core). A per-batch
AllGather (groups [[0..3],[4..7]]) shares the per-head context in
transposed (feature-major) layout. Phase 2 rebuilds hq/ha (Wo + residual +
layernorm, duplicated inside the batch group) and runs block3 for the
core's 2 heads using the rank-1 structure of block3 scores (the
know_params query row is identical for every position, so the first
softmax reduces to prefix sums of one score vector). A second AllGather
shares block3 context; phase 3 produces a [512, 256] output shard
(row quarter q=c%4 of batch b): Wo3 + layernorm + the gated readout.

Device tricks: suffix tensor_tensor_scan for the decay tail (no
cancellation), ACT Exp/Sqrt with per-partition AP scale/bias, scores kept
fp16, dma_start_transpose (XBAR) for all 128x128 transposes, softmax
denominators via ACT accum_out, fp32r matmuls, ACT table-set grouping
(Exp/Sqrt batched over row-tile groups).
"""
import sys

if "/opt/trn_rl_repo" not in sys.path:
    sys.path.insert(0, "/opt/trn_rl_repo")

import numpy as np

import concourse.bacc as bacc
import concourse.mybir as mybir
import concourse.tile as tile
from concourse import bass_utils

dt = mybir.dt
AF = mybir.ActivationFunctionType
ALU = mybir.AluOpType
AX = mybir.AxisListType

S, D, H, DK, B = 2048, 256, 8, 32, 2
NT = S // 128
ISQ = float(1.0 / np.sqrt(DK))
SQ4 = float(DK ** -0.25)   # dk^(-1/4): folded into Wq so QK^T carries 1/sqrt(dk)
GRP = 16
NEG = -30.0

_BUILT = {}


# --------------------------------------------------------------------------
# host-side input preparation (layout + parameter preprocessing only)
# --------------------------------------------------------------------------

def _softplus(x):
    return np.logaddexp(0.0, x)


def build_in_maps(inp):
    f32, f16 = np.float32, np.float16
    q_emb = np.asarray(inp["q_emb"], f32)
    qa_emb = np.asarray(inp["qa_emb"], f32)

    pos16 = np.maximum(
        np.arange(128)[:, None] + 2048 - np.arange(4096)[None, :], 0
    ).astype(f16)
    i_l = np.arange(128)[:, None]
    j_l = np.arange(128)[None, :]
    tri_pk = (j_l > i_l).astype(np.uint8)
    tri_st = (j_l >= i_l).astype(np.uint8)

    know = np.asarray(inp["know_params"], f32)[0, 0]
    q3 = know @ np.asarray(inp["b3_Wq"], f32) + np.asarray(inp["b3_bq"], f32)
    gam = {k: -_softplus(np.asarray(inp[k + "_gam"], f32)[:, 0, 0])
           for k in ("b1", "b2", "b3")}
    Wq = [np.asarray(inp["b1_Wq"], f32), np.asarray(inp["b2_Wq"], f32)]
    Wv = [np.asarray(inp["b1_Wv"], f32), np.asarray(inp["b2_Wv"], f32)]
    bq = [np.asarray(inp["b1_bq"], f32), np.asarray(inp["b2_bq"], f32)]
    bv = [np.asarray(inp["b1_bv"], f32), np.asarray(inp["b2_bv"], f32)]
    Wk3 = np.asarray(inp["b3_Wk"], f32)
    bk3 = np.asarray(inp["b3_bk"], f32)
    lvW = np.asarray(inp["lv_W"], f32)

    def chunk2(w):   # [256, F] -> [128, 2, F]
        return np.ascontiguousarray(w.reshape(2, 128, -1).transpose(1, 0, 2))

    def bc(v):       # [256] -> [128, 256] broadcast
        return np.broadcast_to(np.asarray(v, f32), (128, 256)).copy()

    lvw_pad = np.zeros((128, 8, 256), f16)
    for h in range(8):
        r0 = 32 * (h % 4)
        lvw_pad[r0:r0 + 32, h, :] = lvW.astype(f16)

    maps = []
    for c in range(8):
        b = c // 4
        p = c % 4
        heads = [2 * p, 2 * p + 1]
        X = [q_emb[b], qa_emb[b]]            # [2][2048, 256]

        xT = np.zeros((128, 2, 2, 2048), f32)
        wq_s = np.zeros((128, 2, 2, 2, 32), f32)
        wv_s = np.zeros((128, 2, 2, 64), f32)
        bq_col = np.zeros((32, 2, 2), f32)
        bv_bc = np.zeros((128, 2, 64), f32)
        gam_col = np.zeros((128, 2, 2), f32)
        for blk in range(2):
            xT[:, :, blk, :] = X[blk].T.reshape(2, 128, 2048).transpose(1, 0, 2)
            for hi, h in enumerate(heads):
                hs = slice(32 * h, 32 * h + 32)
                wq_s[:, :, blk, hi, :] = chunk2(Wq[blk][:, hs]) * SQ4
                bq_col[:, blk, hi] = bq[blk][hs] * SQ4
                gam_col[:, blk, hi] = gam[("b1", "b2")[blk]][h]
            wv_s[:, :, blk, :] = chunk2(Wv[blk][:, 64 * p:64 * p + 64])
            bv_bc[:, blk, :] = bv[blk][64 * p:64 * p + 64]

        w16 = np.zeros((128, 2, 2), f16)
        ch_col = np.zeros((1, 2), f32)
        for hi, h in enumerate(heads):
            hs = slice(32 * h, 32 * h + 32)
            w = Wk3[:, hs] @ q3[hs]
            w16[:, :, hi] = w.reshape(2, 128).T.astype(f16)
            ch_col[0, hi] = float((bk3[hs] * q3[hs]).sum() * ISQ)

        qsel = np.zeros((128, 4), f32)
        qsel[:, p] = 1.0

        m = {
            "xT": xT,
            "wq_s": wq_s,
            "bq_col": bq_col,
            "wv_s": wv_s,
            "bv_bc": bv_bc,
            "gam_col": gam_col,
            "pos16": pos16,
            "tri_pk": tri_pk,
            "tri_st": tri_st,
            "r1": q_emb[b] + np.asarray(inp["b1_bo"], f32),
            "r2": qa_emb[b] + np.asarray(inp["b2_bo"], f32),
            "wo1": chunk2(np.asarray(inp["b1_Wo"], f32)),
            "wo2": chunk2(np.asarray(inp["b2_Wo"], f32)),
            "g1_bc": bc(inp["b1_lng"]), "b1_bc": bc(inp["b1_lnb"]),
            "g2_bc": bc(inp["b2_lng"]), "b2_bc": bc(inp["b2_lnb"]),
            "w16": w16,
            "ch_col": ch_col,
            "wv3_s16": chunk2(np.asarray(inp["b3_Wv"], f32)
                              [:, 64 * p:64 * p + 64]).astype(f16),
            "bv3_bc": np.broadcast_to(
                np.asarray(inp["b3_bv"], f32)[64 * p:64 * p + 64],
                (128, 64)).copy(),
            "gam3_col": np.broadcast_to(gam["b3"][heads], (128, 2)).copy(),
            # block3 ctx is gathered per head-pair half (heads 0,2,4,6 then
            # 1,3,5,7) -> permute Wo3 input rows to match
            "wo3": chunk2(np.asarray(inp["b3_Wo"], f32)[
                [32 * h + k for h in (0, 2, 4, 6, 1, 3, 5, 7)
                 for k in range(32)], :]),
            "res3_bc": bc(know + np.asarray(inp["b3_bo"], f32)),
            "g3_bc": bc(inp["b3_lng"]), "b3_bc": bc(inp["b3_lnb"]),
            "lvw_pad16": lvw_pad,
            "lvb_bc": bc(inp["lv_b"]),
            "qrT": np.ascontiguousarray(
                q_emb[b, 512 * p:512 * p + 512].T
                .reshape(2, 128, 512).transpose(1, 0, 2)),
            "kpT": np.ascontiguousarray(know.reshape(8, 32).T),
            "lkw": np.asarray(inp["lk_W"], f32),
            "lkb_col": np.ascontiguousarray(
                np.asarray(inp["lk_b"], f32).reshape(2, 128).T),
            "qsel_col": qsel,
        }
        maps.append(m)
    return maps


# --------------------------------------------------------------------------
# bass program
# --------------------------------------------------------------------------

def _ln(nc, pool, v, g_sb, b_sb, out, tag, eps):
    """out = layernorm(v) * g + b, v fp32 [128, 256]."""
    sv = pool.tile([128, 1], dt.float32, tag=f"sv{tag}")
    nc.vector.tensor_reduce(sv[:], v[:], axis=AX.X, op=ALU.add)
    sq = pool.tile([128, 256], dt.float32, tag=f"sq{tag}")
    s2v = pool.tile([128, 1], dt.float32, tag=f"s2v{tag}")
    nc.scalar.activation(sq[:], v[:], AF.Square, bias=0.0, scale=1.0,
                         accum_out=s2v[:])
    mu = pool.tile([128, 1], dt.float32, tag=f"mu{tag}")
    nc.vector.tensor_scalar(mu[:], sv[:], 1.0 / 256, None, op0=ALU.mult)
    mu2 = pool.tile([128, 1], dt.float32, tag=f"mu2{tag}")
    nc.vector.tensor_tensor(mu2[:], mu[:], mu[:], op=ALU.mult)
    var = pool.tile([128, 1], dt.float32, tag=f"var{tag}")
    nc.vector.scalar_tensor_tensor(var[:], s2v[:], 1.0 / 256, mu2[:],
                                   op0=ALU.mult, op1=ALU.subtract)
    sd = pool.tile([128, 1], dt.float32, tag=f"sd{tag}")
    nc.scalar.activation(sd[:], var[:], AF.Sqrt, bias=eps[:], scale=1.0)
    rstd = pool.tile([128, 1], dt.float32, tag=f"rstd{tag}")
    nc.vector.reciprocal(rstd[:], sd[:])
    xn = pool.tile([128, 256], dt.float32, tag=f"xn{tag}")
    nc.vector.tensor_scalar(xn[:], v[:], mu[:], rstd[:],
                            op0=ALU.subtract, op1=ALU.mult)
    nc.vector.tensor_tensor(xn[:], xn[:], g_sb[:], op=ALU.mult)
    nc.vector.tensor_tensor(out, xn[:], b_sb[:], op=ALU.add)


def _attn_triangle(nc, wp, ctxps, sc_mm, gam_ap, Vtile, v_off,
                   pos, tri, neg16, agdst, agrow, strict, sbc=None,
                   Pbc=None, ninvP=None, ones16=None, grp=GRP):
    """The per-unit decay-bias attention triangle (16 row tiles, grouped).

    Blocks 1/2 path: sc_mm(t, f, fw, tag) emits the scores matmul chunk
    into PSUM and returns the psum tile (stage A reads it through Exp,
    stage D re-issues it for the s2 product — cheaper than keeping an
    fp16 score copy in SBUF). Block3 path (strict=True, sbc/Pbc given):
    rank-1 scores, no matmul/scan. tail/t1 are sums/ratios of
    exponentials so the reference's max(.,0) clamps are dead — plain
    tensor_tensor mult gets the 2x fp16 DVE mode.
    """
    for g0 in range(0, NT, grp):
        tiles = list(range(g0, min(g0 + grp, NT)))
        chain = {}
        if not strict:
            # stage A: scores -> e (Exp); 1/sqrt(dk) is folded into Wq
            for t in tiles:
                W = 128 * (t + 1)
                e = wp.tile([128, 2048], dt.float16, tag=f"ch{t - g0}", name="e")
                chain[t] = e
                for f in range((W + 511) // 512):
                    fw = min(512, W - 512 * f)
                    ps = sc_mm(t, f, fw, "sc")
                    nc.scalar.activation(e[:, 512 * f:512 * f + fw],
                                         ps[:, :fw], AF.Exp, bias=0.0,
                                         scale=1.0)
                nc.vector.copy_predicated(e[:, W - 128:W], tri[:], neg16[1])
            # stage B: suffix scan -> tail, u = tail*pos
            invZ = wp.tile([128, grp], dt.float32, tag="invZ")
            usb = {}
            for t in tiles:
                W = 128 * (t + 1)
                suf = wp.tile([128, 2049], dt.float16, tag=f"sf{t - g0}", name="suf")
                nc.vector.memset(suf[:, W:W + 1], 0.0)
                nc.vector.tensor_tensor_scan(
                    suf[:, :W][:, ::-1], chain[t][:, :W][:, ::-1],
                    chain[t][:, :W][:, ::-1], 0.0, op0=ALU.add, op1=ALU.bypass)
                nc.vector.reciprocal(invZ[:, t - g0:t - g0 + 1], suf[:, 0:1])
                u = wp.tile([128, 2048], dt.float16, tag=f"ch{t - g0}", name="u")
                nc.vector.tensor_tensor(
                    u[:, :W], suf[:, 1:W + 1],
                    pos[:, 2048 - 128 * t:2048 - 128 * t + W], op=ALU.mult)
                usb[t] = u
        else:
            # block3 stage B': t1 = P*(-1/Pprev) + 1, u = t1*pos
            usb = {}
            invZ = None
            for t in tiles:
                W = 128 * (t + 1)
                t1 = wp.tile([128, 2048], dt.float16, tag=f"sf{t - g0}", name="t1")
                nc.vector.scalar_tensor_tensor(
                    t1[:, :W], Pbc[:, :W], ninvP[:, t:t + 1], ones16[:, :W],
                    op0=ALU.mult, op1=ALU.add)
                u = wp.tile([128, 2048], dt.float16, tag=f"ch{t - g0}", name="u")
                nc.vector.tensor_tensor(
                    u[:, :W], t1[:, :W],
                    pos[:, 2048 - 128 * t:2048 - 128 * t + W], op=ALU.mult)
                usb[t] = u
        # stage C: r = sqrt(u * invZ)  (invZ=1 for block3)
        rsb = {}
        for t in tiles:
            W = 128 * (t + 1)
            r = wp.tile([128, 2048], dt.float16, tag=f"sf{t - g0}", name="r")
            if strict:
                nc.scalar.activation(r[:, :W], usb[t][:, :W], AF.Sqrt,
                                     bias=0.0, scale=1.0)
            else:
                nc.scalar.activation(r[:, :W], usb[t][:, :W], AF.Sqrt,
                                     bias=0.0, scale=invZ[:, t - g0:t - g0 + 1])
            rsb[t] = r
        # stage D: eff, scores2, e2, p2, transpose, ctx matmul, store
        for t in tiles:
            W = 128 * (t + 1)
            m0 = wp.tile([128, 2048], dt.float16, tag=f"ch{t - g0}", name="m0")
            nc.scalar.activation(m0[:, :W], rsb[t][:, :W], AF.Exp,
                                 bias=0.0, scale=gam_ap)
            s2 = wp.tile([128, 2048], dt.float16, tag=f"sf{t - g0}", name="s2")
            if strict:
                nc.vector.tensor_tensor(s2[:, :W], m0[:, :W], sbc[:, :W],
                                        op=ALU.mult)
            else:
                for f in range((W + 511) // 512):
                    fw = min(512, W - 512 * f)
                    ps = sc_mm(t, f, fw, "sc2")
                    nc.vector.tensor_tensor(
                        s2[:, 512 * f:512 * f + fw],
                        m0[:, 512 * f:512 * f + fw], ps[:, :fw], op=ALU.mult)
            nc.vector.copy_predicated(s2[:, W - 128:W], tri[:], neg16[0])
            e2 = wp.tile([128, 2048], dt.float16, tag=f"ch{t - g0}", name="e2")
            Z2 = wp.tile([128, 1], dt.float32, tag="Z2")
            nc.scalar.activation(e2[:, :W], s2[:, :W], AF.Exp, bias=0.0,
                                 scale=1.0, accum_out=Z2[:])
            iZ2 = wp.tile([128, 1], dt.float32, tag="iZ2")
            nc.vector.reciprocal(iZ2[:], Z2[:])
            p2 = wp.tile([128, 2048], dt.float16, tag=f"sf{t - g0}", name="p2")
            nc.vector.tensor_scalar(p2[:, :W], e2[:, :W], iZ2[:], None,
                                    op0=ALU.mult)
            if strict and t == 0:
                nc.vector.memset(p2[0:1, 0:128], 0.0)
            p2T = wp.tile([128, 16, 128], dt.float16, tag=f"ch{t - g0}", name="p2T")
            nc.sync.dma_start_transpose(p2T[:, :t + 1, :], p2[:, :W])
            cps = ctxps.tile([32, 128], dt.float32, tag="ctx")
            for jb in range(t + 1):
                nc.tensor.matmul(cps[:], Vtile[:, jb, v_off:v_off + 32],
                                 p2T[:, jb, :], start=(jb == 0), stop=(jb == t))
            csb = wp.tile([32, 128], dt.float32, tag="csb", name="csb")
            nc.vector.tensor_copy(csb[:], cps[:])
            nc.sync.dma_start(
                agdst[agrow:agrow + 32, 128 * t:128 * t + 128], csb[:])


def build_bass(sim=False):
    nc = bacc.Bacc("TRN2", target_bir_lowering=False)

    def din(name, shape, dtyp=dt.float32):
        return nc.dram_tensor(name, shape, dtyp, kind="ExternalInput")

    tin = {
        "xT": din("xT", [128, 2, 2, 2048], dt.float32r),
        "wq_s": din("wq_s", [128, 2, 2, 2, 32], dt.float32r),
        "bq_col": din("bq_col", [32, 2, 2]),
        "wv_s": din("wv_s", [128, 2, 2, 64], dt.float32r),
        "bv_bc": din("bv_bc", [128, 2, 64]),
        "gam_col": din("gam_col", [128, 2, 2]),
        "pos16": din("pos16", [128, 4096], dt.float16),
        "tri_pk": din("tri_pk", [128, 128], dt.uint8),
        "tri_st": din("tri_st", [128, 128], dt.uint8),
        "r1": din("r1", [2048, 256]),
        "r2": din("r2", [2048, 256]),
        "wo1": din("wo1", [128, 2, 256], dt.float32r),
        "wo2": din("wo2", [128, 2, 256], dt.float32r),
        "g1_bc": din("g1_bc", [128, 256]), "b1_bc": din("b1_bc", [128, 256]),
        "g2_bc": din("g2_bc", [128, 256]), "b2_bc": din("b2_bc", [128, 256]),
        "w16": din("w16", [128, 2, 2], dt.float16),
        "ch_col": din("ch_col", [1, 2]),
        "wv3_s16": din("wv3_s16", [128, 2, 64], dt.float16),
        "bv3_bc": din("bv3_bc", [128, 64]),
        "gam3_col": din("gam3_col", [128, 2]),
        "wo3": din("wo3", [128, 2, 256]),
        "res3_bc": din("res3_bc", [128, 256]),
        "g3_bc": din("g3_bc", [128, 256]), "b3_bc": din("b3_bc", [128, 256]),
        "lvw_pad16": din("lvw_pad16", [128, 8, 256], dt.float16),
        "lvb_bc": din("lvb_bc", [128, 256]),
        "qrT": din("qrT", [128, 2, 512]),
        "kpT": din("kpT", [32, 8]),
        "lkw": din("lkw", [32, 256]),
        "lkb_col": din("lkb_col", [128, 2]),
        "qsel_col": din("qsel_col", [128, 4]),
    }
    out_t = nc.dram_tensor("out", [512, 256], dt.float32,
                           kind="ExternalOutput")

    with tile.TileContext(nc) as tc, \
         tc.tile_pool(name="consts", bufs=1) as cs, \
         tc.tile_pool(name="dram", bufs=1, space="DRAM") as dram:
        pos = cs.tile([128, 4096], dt.float16)
        nc.sync.dma_start(pos[:], tin["pos16"][:])
        tpk = cs.tile([128, 128], dt.uint8)
        nc.sync.dma_start(tpk[:], tin["tri_pk"][:])
        tst = cs.tile([128, 128], dt.uint8)
        nc.sync.dma_start(tst[:], tin["tri_st"][:])
        gamc = cs.tile([128, 2, 2], dt.float32)
        nc.sync.dma_start(gamc[:], tin["gam_col"][:])
        gam3c = cs.tile([128, 2], dt.float32)
        nc.sync.dma_start(gam3c[:], tin["gam3_col"][:])
        bqc = cs.tile([32, 2, 2], dt.float32)
        nc.sync.dma_start(bqc[:], tin["bq_col"][:])
        bvb = cs.tile([128, 2, 64], dt.float32)
        nc.sync.dma_start(bvb[:], tin["bv_bc"][:])
        neg16 = cs.tile([128, 128], dt.float16)
        nc.vector.memset(neg16[:], NEG)
        zero16 = cs.tile([128, 128], dt.float16)
        nc.vector.memset(zero16[:], 0.0)
        ones16 = cs.tile([128, 2048], dt.float16)
        nc.vector.memset(ones16[:], 1.0)
        tch = cs.tile([1, 4], dt.float16)
        nc.vector.tensor_copy(tch[:, 0:1], pos[:1, :1])
        epsc = cs.tile([128, 1], dt.float32)
        nc.vector.memset(epsc[:], 1e-5)

        # per-block / per-head-pair gather halves so each collective can
        # overlap the next unit's compute instead of stalling the core
        agin1a = dram.tile([64, 2048], dt.float32)
        agout1a = dram.tile([256, 2048], dt.float32)
        agin1b = dram.tile([64, 2048], dt.float32)
        agout1b = dram.tile([256, 2048], dt.float32)
        agin2a = dram.tile([32, 2048], dt.float32)
        agout2a = dram.tile([128, 2048], dt.float32)
        agin2b = dram.tile([32, 2048], dt.float32)
        agout2b = dram.tile([128, 2048], dt.float32)
        pbuf = dram.tile([2, 2049], dt.float32)

        # ---------------- phase 1 ----------------
        qtv_cm = tc.tile_pool(name="qtv", bufs=1)
        qtv = qtv_cm.__enter__()
        QT = qtv.tile([32, 2, 2, 2048], dt.float16)
        V16 = qtv.tile([128, 2, 16, 64], dt.float16)
        with tc.tile_pool(name="proj", bufs=1) as pj, \
             tc.tile_pool(name="pjps", bufs=2, space="PSUM") as pjps:
            xTs = pj.tile([128, 2, 2, 2048], dt.float32r)
            nc.sync.dma_start(xTs[:], tin["xT"][:])
            wqs = pj.tile([128, 2, 2, 2, 32], dt.float32r)
            nc.sync.dma_start(wqs[:], tin["wq_s"][:])
            wvs = pj.tile([128, 2, 2, 64], dt.float32r)
            nc.sync.dma_start(wvs[:], tin["wv_s"][:])
            for blk in range(2):
                for hi in range(2):
                    for f in range(4):
                        ps = pjps.tile([32, 512], dt.float32, tag="qt")
                        for cch in range(2):
                            nc.tensor.matmul(
                                ps[:], wqs[:, cch, blk, hi, :],
                                xTs[:, cch, blk, 512 * f:512 * f + 512],
                                start=(cch == 0), stop=(cch == 1))
                        nc.scalar.activation(
                            QT[:, blk, hi, 512 * f:512 * f + 512], ps[:],
                            AF.Identity, bias=bqc[:, blk, hi:hi + 1],
                            scale=1.0)
                for jb in range(16):
                    ps = pjps.tile([128, 64], dt.float32, tag="v")
                    for cch in range(2):
                        nc.tensor.matmul(
                            ps[:], xTs[:, cch, blk, 128 * jb:128 * jb + 128],
                            wvs[:, cch, blk, :], start=(cch == 0),
                            stop=(cch == 1))
                    nc.vector.tensor_tensor(V16[:, blk, jb, :], ps[:],
                                            bvb[:, blk, :], op=ALU.add)

        with tc.tile_pool(name="p1", bufs=1) as wp, \
             tc.tile_pool(name="scps", bufs=3, space="PSUM") as scps, \
             tc.tile_pool(name="ctxps", bufs=2, space="PSUM") as ctxps:
            for blk in range(2):
                agdst = (agin1a, agin1b)[blk]
                for hi in range(2):
                    def sc_mm(t, f, fw, tag, blk=blk, hi=hi):
                        ps = scps.tile([128, 512], dt.float32, tag=tag)
                        nc.tensor.matmul(
                            ps[:, :fw],
                            QT[:, blk, hi, 128 * t:128 * t + 128],
                            QT[:, blk, hi, 512 * f:512 * f + fw],
                            start=True, stop=True)
                        return ps
                    _attn_triangle(
                        nc, wp, ctxps, sc_mm,
                        gamc[:, blk, hi:hi + 1], V16[:, blk], 32 * hi,
                        pos, tpk, (neg16, zero16), agdst,
                        32 * hi, strict=False)
                # gather this block's context now; overlaps the next
                # block's triangles / the hq rebuild
                agout = (agout1a, agout1b)[blk]
                if sim:
                    for rr in range(4):
                        nc.gpsimd.dma_start(agout[64 * rr:64 * rr + 64, :],
                                            agdst[:])
                else:
                    nc.gpsimd.collective_compute(
                        "AllGather", ALU.bypass, ins=[agdst.opt()],
                        outs=[agout.opt()],
                        replica_groups=[[0, 1, 2, 3], [4, 5, 6, 7]])

        qtv_cm.__exit__(None, None, None)

        # ---------------- phase 2: hq/ha ----------------
        hq16 = cs.tile([128, 2, 2048], dt.float16)
        ha16 = cs.tile([128, 2, 2048], dt.float16)
        with tc.tile_pool(name="p2", bufs=2) as p2, \
             tc.tile_pool(name="lnp", bufs=4) as lnp, \
             tc.tile_pool(name="p2ps", bufs=3, space="PSUM") as p2ps:
            for which, (wo_n, res_n, g_n, bb_n, dstT, agsrc) in enumerate([
                    ("wo1", "r1", "g1_bc", "b1_bc", hq16, agout1a),
                    ("wo2", "r2", "g2_bc", "b2_bc", ha16, agout1b)]):
                wo_sb = p2.tile([128, 2, 256], dt.float32r, tag="wo")
                nc.sync.dma_start(wo_sb[:], tin[wo_n][:])
                g_sb = p2.tile([128, 256], dt.float32, tag="g")
                nc.sync.dma_start(g_sb[:], tin[g_n][:])
                bb_sb = p2.tile([128, 256], dt.float32, tag="bb")
                nc.sync.dma_start(bb_sb[:], tin[bb_n][:])
                ctxT = p2.tile([128, 2, 2048], dt.float32r, tag="ctxT")
                for cch in range(2):
                    nc.gpsimd.dma_start(
                        ctxT[:, cch, :],
                        agsrc[128 * cch:128 * cch + 128, :])
                h16 = p2.tile([128, 16, 256], dt.float16, tag="h16")
                for ic in range(16):
                    ps = p2ps.tile([128, 256], dt.float32, tag="wops")
                    for cch in range(2):
                        nc.tensor.matmul(
                            ps[:], ctxT[:, cch, 128 * ic:128 * ic + 128],
                            wo_sb[:, cch, :], start=(cch == 0),
                            stop=(cch == 1))
                    res_sb = lnp.tile([128, 256], dt.float32, tag="res")
                    nc.sync.dma_start(res_sb[:],
                                      tin[res_n][128 * ic:128 * ic + 128, :])
                    v = lnp.tile([128, 256], dt.float32, tag="v")
                    nc.vector.tensor_tensor(v[:], ps[:], res_sb[:], op=ALU.add)
                    _ln(nc, lnp, v, g_sb, bb_sb, h16[:, ic, :], "a", epsc)
                    nc.sync.dma_start_transpose(
                        dstT[:, :, 128 * ic:128 * ic + 128], h16[:, ic, :])

        # ---------------- phase 2: block3 ----------------
        with tc.tile_pool(name="b3", bufs=1) as wp, \
             tc.tile_pool(name="b3ps", bufs=2, space="PSUM") as ps3, \
             tc.tile_pool(name="b3ctx", bufs=2, space="PSUM") as ctxps:
            w16sb = wp.tile([128, 2, 2], dt.float16, tag="w16")
            nc.sync.dma_start(w16sb[:], tin["w16"][:])
            chc = wp.tile([1, 2], dt.float32, tag="chc")
            nc.sync.dma_start(chc[:], tin["ch_col"][:])
            wv3 = wp.tile([128, 2, 64], dt.float16, tag="wv3")
            nc.sync.dma_start(wv3[:], tin["wv3_s16"][:])
            bv3 = wp.tile([128, 64], dt.float32, tag="bv3")
            nc.sync.dma_start(bv3[:], tin["bv3_bc"][:])
            one1 = wp.tile([1, 1], dt.float32, tag="one1")
            nc.vector.memset(one1[:], 1.0)
            onesrow = wp.tile([1, 128], dt.float32, tag="onesrow")
            nc.vector.memset(onesrow[:], 1.0)

            V3 = wp.tile([128, 16, 64], dt.float16, tag="V3")
            for jb in range(16):
                vps = ps3.tile([128, 64], dt.float32, tag="v3")
                for cch in range(2):
                    nc.tensor.matmul(
                        vps[:], ha16[:, cch, 128 * jb:128 * jb + 128],
                        wv3[:, cch, :], start=(cch == 0), stop=(cch == 1))
                nc.vector.tensor_tensor(V3[:, jb, :], vps[:], bv3[:],
                                        op=ALU.add)

            for hi in range(2):
                s_row = wp.tile([1, 2048], dt.float32, tag="srow")
                for f in range(4):
                    sp = ps3.tile([1, 512], dt.float32, tag="s")
                    for cch in range(2):
                        nc.tensor.matmul(
                            sp[:], w16sb[:, cch, hi:hi + 1],
                            hq16[:, cch, 512 * f:512 * f + 512],
                            start=(cch == 0), stop=(cch == 1))
                    nc.scalar.activation(
                        s_row[:, 512 * f:512 * f + 512], sp[:], AF.Identity,
                        bias=chc[:, hi:hi + 1], scale=ISQ)
                smax = wp.tile([1, 1], dt.float32, tag="smax")
                nc.vector.tensor_reduce(smax[:], s_row[:], axis=AX.X,
                                        op=ALU.max)
                nsmax = wp.tile([1, 1], dt.float32, tag="nsmax")
                nc.vector.tensor_scalar(nsmax[:], smax[:], -1.0, None,
                                        op0=ALU.mult)
                e3 = wp.tile([1, 2048], dt.float32, tag="e3")
                nc.scalar.activation(e3[:], s_row[:], AF.Exp, bias=nsmax[:],
                                     scale=1.0)
                P_row = wp.tile([1, 2048], dt.float32, tag="Prow")
                nc.vector.tensor_tensor_scan(P_row[:], e3[:], e3[:], 0.0,
                                             op0=ALU.add, op1=ALU.bypass)
                nc.sync.dma_start(pbuf[hi, 0:1], one1[:])
                nc.sync.dma_start(pbuf[hi, 1:2049], P_row[:])
                npcol = wp.tile([128, 16], dt.float32, tag="npcol")
                pcol = wp.tile([128, 16], dt.float32, tag="pcol")
                nc.sync.dma_start(
                    pcol[:], pbuf[hi, 0:2048].rearrange("(t p) -> p t", p=128))
                nc.vector.tensor_scalar(pcol[:], pcol[:], -1.0, None,
                                        op0=ALU.mult)
                nc.vector.reciprocal(npcol[:], pcol[:])
                P_bc = wp.tile([128, 2048], dt.float32, tag="Pbc")
                s_bc = wp.tile([128, 2048], dt.float16, tag="sbc")
                for f in range(4):
                    bp = ps3.tile([128, 512], dt.float32, tag="bc")
                    nc.tensor.matmul(bp[:], onesrow[:],
                                     P_row[:, 512 * f:512 * f + 512],
                                     start=True, stop=True)
                    nc.vector.tensor_copy(P_bc[:, 512 * f:512 * f + 512],
                                          bp[:])
                    bs = ps3.tile([128, 512], dt.float32, tag="bc")
                    nc.tensor.matmul(bs[:], onesrow[:],
                                     s_row[:, 512 * f:512 * f + 512],
                                     start=True, stop=True)
                    nc.vector.tensor_copy(s_bc[:, 512 * f:512 * f + 512],
                                          bs[:])
                agdst = (agin2a, agin2b)[hi]
                _attn_triangle(
                    nc, wp, ctxps, None, gam3c[:, hi:hi + 1],
                    V3, 32 * hi, pos, tst, (neg16, zero16), agdst, 0,
                    strict=True, sbc=s_bc, Pbc=P_bc, ninvP=npcol,
                    ones16=ones16)
                agout = (agout2a, agout2b)[hi]
                if sim:
                    for rr in range(4):
                        nc.gpsimd.dma_start(
                            agout[32 * rr:32 * rr + 32, :], agdst[:])
                else:
                    nc.gpsimd.collective_compute(
                        "AllGather", ALU.bypass, ins=[agdst.opt()],
                        outs=[agout.opt()],
                        replica_groups=[[0, 1, 2, 3], [4, 5, 6, 7]])

        # ---------------- phase 3 ----------------
        with tc.tile_pool(name="p3", bufs=2) as wp, \
             tc.tile_pool(name="p3ps", bufs=2, space="PSUM") as ps:
            wo3 = wp.tile([128, 2, 256], dt.float32, tag="wo3")
            nc.sync.dma_start(wo3[:], tin["wo3"][:])
            res3 = wp.tile([128, 256], dt.float32, tag="res3")
            nc.sync.dma_start(res3[:], tin["res3_bc"][:])
            g3 = wp.tile([128, 256], dt.float32, tag="g3")
            nc.sync.dma_start(g3[:], tin["g3_bc"][:])
            b3 = wp.tile([128, 256], dt.float32, tag="b3")
            nc.sync.dma_start(b3[:], tin["b3_bc"][:])
            lvw = wp.tile([128, 8, 256], dt.float16, tag="lvw")
            nc.sync.dma_start(lvw[:], tin["lvw_pad16"][:])
            lvb = wp.tile([128, 256], dt.float32, tag="lvb")
            nc.sync.dma_start(lvb[:], tin["lvb_bc"][:])
            qrTs = wp.tile([128, 2, 512], dt.float32, tag="qrTs")
            nc.sync.dma_start(qrTs[:], tin["qrT"][:])
            kpTs = wp.tile([32, 8], dt.float32, tag="kpTs")
            nc.sync.dma_start(kpTs[:], tin["kpT"][:])
            lkws = wp.tile([32, 256], dt.float32, tag="lkws")
            nc.sync.dma_start(lkws[:], tin["lkw"][:])
            lkbc = wp.tile([128, 2], dt.float32, tag="lkbc")
            nc.sync.dma_start(lkbc[:], tin["lkb_col"][:])
            qsel = wp.tile([128, 4], dt.float32, tag="qsel")
            nc.sync.dma_start(qsel[:], tin["qsel_col"][:])

            keyT = wp.tile([128, 2, 8], dt.float32, tag="keyT")
            for cch in range(2):
                kps = ps.tile([128, 8], dt.float32, tag="key")
                nc.tensor.matmul(kps[:], lkws[:, 128 * cch:128 * cch + 128],
                                 kpTs[:], start=True, stop=True)
                nc.scalar.activation(keyT[:, cch, :], kps[:], AF.Sigmoid,
                                     bias=lkbc[:, cch:cch + 1], scale=1.0)

            ag2f = wp.tile([128, 2, 2048], dt.float32, tag="ag2f")
            for cch, agsrc in enumerate((agout2a, agout2b)):
                nc.sync.dma_start(ag2f[:, cch, :], agsrc[:])
            # select this core's column quarter via the one-hot qsel blend
            ag2sb = wp.tile([128, 2, 512], dt.float32, tag="ag2sb")
            for cch in range(2):
                blendt = wp.tile([128, 512], dt.float32, tag="blendt")
                nc.vector.tensor_scalar(
                    blendt[:], ag2f[:, cch, 0:512], qsel[:, 0:1], None,
                    op0=ALU.mult)
                for qq in range(1, 4):
                    dst = blendt if qq < 3 else None
                    if qq < 3:
                        nc.vector.scalar_tensor_tensor(
                            blendt[:], ag2f[:, cch, 512 * qq:512 * qq + 512],
                            qsel[:, qq:qq + 1], blendt[:],
                            op0=ALU.mult, op1=ALU.add)
                    else:
                        nc.vector.scalar_tensor_tensor(
                            ag2sb[:, cch, :],
                            ag2f[:, cch, 512 * qq:512 * qq + 512],
                            qsel[:, qq:qq + 1], blendt[:],
                            op0=ALU.mult, op1=ALU.add)

            # stage-major over the 4 row tiles so ACT runs each function
            # (sqrt-table LN, Exp, Sigmoid) as one batch - 3 table loads
            # instead of ~4 per tile
            h3Ts, alphas = [], []
            for ic in range(4):
                wops = ps.tile([128, 256], dt.float32, tag="wo3ps")
                for cch in range(2):
                    nc.tensor.matmul(
                        wops[:], ag2sb[:, cch, 128 * ic:128 * ic + 128],
                        wo3[:, cch, :], start=(cch == 0), stop=(cch == 1))
                v = wp.tile([128, 256], dt.float32, tag=f"v3p{ic}")
                nc.vector.tensor_tensor(v[:], wops[:], res3[:], op=ALU.add)
                h3 = wp.tile([128, 256], dt.float32, tag=f"h3{ic}")
                _ln(nc, wp, v, g3, b3, h3[:], f"3{ic}", epsc)
                h316 = wp.tile([128, 256], dt.float16, tag=f"h316{ic}")
                nc.vector.tensor_copy(h316[:], h3[:])
                h3T = wp.tile([128, 2, 128], dt.float16, tag=f"h3T{ic}")
                nc.sync.dma_start_transpose(h3T[:], h316[:])
                h3Ts.append(h3T)
            for ic in range(4):
                bps = ps.tile([128, 8], dt.float32, tag="beta")
                for cch in range(2):
                    nc.tensor.matmul(
                        bps[:], qrTs[:, cch, 128 * ic:128 * ic + 128],
                        keyT[:, cch, :], start=(cch == 0), stop=(cch == 1))
                bmax = wp.tile([128, 1], dt.float32, tag=f"bmax{ic}")
                nc.vector.tensor_reduce(bmax[:], bps[:], axis=AX.X,
                                        op=ALU.max)
                nbmax = wp.tile([128, 1], dt.float32, tag=f"nbmax{ic}")
                nc.vector.tensor_scalar(nbmax[:], bmax[:], -1.0, None,
                                        op0=ALU.mult)
                ebeta = wp.tile([128, 8], dt.float32, tag=f"ebeta{ic}")
                zb = wp.tile([128, 1], dt.float32, tag=f"zb{ic}")
                nc.scalar.activation(ebeta[:], bps[:], AF.Exp, bias=nbmax[:],
                                     scale=1.0, accum_out=zb[:])
                izb = wp.tile([128, 1], dt.float32, tag=f"izb{ic}")
                nc.vector.reciprocal(izb[:], zb[:])
                alpha = wp.tile([128, 8], dt.float32, tag=f"alpha{ic}")
                nc.vector.tensor_scalar(alpha[:], ebeta[:], izb[:], None,
                                        op0=ALU.mult)
                alphas.append(alpha)
            for ic in range(4):
                h3T, alpha = h3Ts[ic], alphas[ic]
                acc = wp.tile([128, 256], dt.float32, tag=f"acc{ic}")
                accb = wp.tile([128, 256], dt.float32, tag=f"accb{ic}")
                nc.vector.memset(acc[:], 0.0)
                for h in range(8):
                    vps = ps.tile([128, 256], dt.float32, tag="valps")
                    nc.tensor.matmul(vps[:], h3T[:, h // 4, :], lvw[:, h, :],
                                     start=True, stop=True)
                    val = wp.tile([128, 256], dt.float32, tag="val")
                    nc.vector.tensor_tensor(val[:], vps[:], lvb[:],
                                            op=ALU.add)
                    vsg = wp.tile([128, 256], dt.float32, tag="vsg")
                    nc.scalar.activation(vsg[:], val[:], AF.Sigmoid,
                                         bias=0.0, scale=1.0)
                    src, dst2 = (acc, accb) if h % 2 == 0 else (accb, acc)
                    nc.vector.scalar_tensor_tensor(
                        dst2[:], vsg[:], alpha[:, h:h + 1], src[:],
                        op0=ALU.mult, op1=ALU.add)
                nc.sync.dma_start(out_t[128 * ic:128 * ic + 128, :], acc[:])

    nc.finalize()
    return nc


def run(inputs, **kw):
    if "nc" not in _BUILT:
        _BUILT["nc"] = build_bass()
    nc = _BUILT["nc"]
    in_maps = build_in_maps(inputs)
    res = bass_utils.run_bass_kernel_spmd(nc, in_maps,
                                          core_ids=list(range(8)), **kw)
    out = np.zeros((2, 2048, 256), np.float32)
    for c in range(8):
        b, q = c // 4, c % 4
        out[b, 512 * q:512 * q + 512, :] = res.results[c]["out"]
    return out, res


def kernel(**inputs):
    return run(inputs)[0]



# revision 24
# speedup vs baseline: 1.2556x; 1.0557x over previous
"""DTransformer forward on 8 trn2 NeuronCores (bass/Tile, single launch).

Sharding: core c handles batch b=c//4 and head pair p=c%4 (heads 2p, 2p+1)
of ALL three attention blocks. Phase 1 computes blocks 1&2 per-head
attention with the distance-decay bias (4 units/core). A per-batch
AllGather (groups [[0..3],[4..7]]) shares the per-head context in
transposed (feature-major) layout. Phase 2 rebuilds hq/ha (Wo + residual +
layernorm, duplicated inside the batch group) and runs block3 for the
core's 2 heads using the rank-1 structure of block3 scores (the
know_params query row is identical for every position, so the first
softmax reduces to prefix sums of one score vector). A second AllGather
shares block3 context; phase 3 produces a [512, 256] output shard
(row quarter q=c%4 of batch b): Wo3 + layernorm + the gated readout.

Device tricks: suffix tensor_tensor_scan for the decay tail (no
cancellation), ACT Exp/Sqrt with per-partition AP scale/bias, scores kept
fp16, dma_start_transpose (XBAR) for all 128x128 transposes, softmax
denominators via ACT accum_out, fp32r matmuls, ACT table-set grouping
(Exp/Sqrt batched over row-tile groups).
"""
import sys

if "/opt/trn_rl_repo" not in sys.path:
    sys.path.insert(0, "/opt/trn_rl_repo")

import numpy as np

import concourse.bacc as bacc
import concourse.mybir as mybir
import concourse.tile as tile
from concourse import bass_utils

dt = mybir.dt
AF = mybir.ActivationFunctionType
ALU = mybir.AluOpType
AX = mybir.AxisListType

S, D, H, DK, B = 2048, 256, 8, 32, 2
NT = S // 128
ISQ = float(1.0 / np.sqrt(DK))
SQ4 = float(DK ** -0.25)   # dk^(-1/4): folded into Wq so QK^T carries 1/sqrt(dk)
GRP = 16
NEG = -30.0

_BUILT = {}


# --------------------------------------------------------------------------
# host-side input preparation (layout + parameter preprocessing only)
# --------------------------------------------------------------------------

def _softplus(x):
    return np.logaddexp(0.0, x)


def build_in_maps(inp):
    f32, f16 = np.float32, np.float16
    q_emb = np.asarray(inp["q_emb"], f32)
    qa_emb = np.asarray(inp["qa_emb"], f32)

    pos16 = np.maximum(
        np.arange(128)[:, None] + 2048 - np.arange(4096)[None, :], 0
    ).astype(f16)
    i_l = np.arange(128)[:, None]
    j_l = np.arange(128)[None, :]
    tri_pk = (j_l > i_l).astype(np.uint8)
    tri_st = (j_l >= i_l).astype(np.uint8)

    know = np.asarray(inp["know_params"], f32)[0, 0]
    q3 = know @ np.asarray(inp["b3_Wq"], f32) + np.asarray(inp["b3_bq"], f32)
    gam = {k: -_softplus(np.asarray(inp[k + "_gam"], f32)[:, 0, 0])
           for k in ("b1", "b2", "b3")}
    Wq = [np.asarray(inp["b1_Wq"], f32), np.asarray(inp["b2_Wq"], f32)]
    Wv = [np.asarray(inp["b1_Wv"], f32), np.asarray(inp["b2_Wv"], f32)]
    bq = [np.asarray(inp["b1_bq"], f32), np.asarray(inp["b2_bq"], f32)]
    bv = [np.asarray(inp["b1_bv"], f32), np.asarray(inp["b2_bv"], f32)]
    Wk3 = np.asarray(inp["b3_Wk"], f32)
    bk3 = np.asarray(inp["b3_bk"], f32)
    lvW = np.asarray(inp["lv_W"], f32)

    def chunk2(w):   # [256, F] -> [128, 2, F]
        return np.ascontiguousarray(w.reshape(2, 128, -1).transpose(1, 0, 2))

    def bc(v):       # [256] -> [128, 256] broadcast
        return np.broadcast_to(np.asarray(v, f32), (128, 256)).copy()

    lvw_pad = np.zeros((128, 8, 256), f16)
    for h in range(8):
        r0 = 32 * (h % 4)
        lvw_pad[r0:r0 + 32, h, :] = lvW.astype(f16)

    maps = []
    for c in range(8):
        b = c // 4
        p = c % 4
        heads = [2 * p, 2 * p + 1]
        X = [q_emb[b], qa_emb[b]]            # [2][2048, 256]

        xT = np.zeros((128, 2, 2, 2048), f32)
        wq_s = np.zeros((128, 2, 2, 2, 32), f32)
        wv_s = np.zeros((128, 2, 2, 64), f32)
        bq_col = np.zeros((32, 2, 2), f32)
        bv_bc = np.zeros((128, 2, 64), f32)
        gam_col = np.zeros((128, 2, 2), f32)
        for blk in range(2):
            xT[:, :, blk, :] = X[blk].T.reshape(2, 128, 2048).transpose(1, 0, 2)
            for hi, h in enumerate(heads):
                hs = slice(32 * h, 32 * h + 32)
                wq_s[:, :, blk, hi, :] = chunk2(Wq[blk][:, hs]) * SQ4
                bq_col[:, blk, hi] = bq[blk][hs] * SQ4
                gam_col[:, blk, hi] = gam[("b1", "b2")[blk]][h]
            wv_s[:, :, blk, :] = chunk2(Wv[blk][:, 64 * p:64 * p + 64])
            bv_bc[:, blk, :] = bv[blk][64 * p:64 * p + 64]

        w16 = np.zeros((128, 2, 2), f16)
        ch_col = np.zeros((1, 2), f32)
        for hi, h in enumerate(heads):
            hs = slice(32 * h, 32 * h + 32)
            w = Wk3[:, hs] @ q3[hs]
            w16[:, :, hi] = w.reshape(2, 128).T.astype(f16)
            ch_col[0, hi] = float((bk3[hs] * q3[hs]).sum() * ISQ)

        qsel = np.zeros((128, 4), f32)
        qsel[:, p] = 1.0

        m = {
            "xT": xT,
            "wq_s": wq_s,
            "bq_col": bq_col,
            "wv_s": wv_s,
            "bv_bc": bv_bc,
            "gam_col": gam_col,
            "pos16": pos16,
            "tri_pk": tri_pk,
            "tri_st": tri_st,
            "r1q": (q_emb[b] + np.asarray(inp["b1_bo"], f32))
                   [512 * p:512 * p + 512],
            "r2q": (qa_emb[b] + np.asarray(inp["b2_bo"], f32))
                   [512 * p:512 * p + 512],
            "wo1": chunk2(np.asarray(inp["b1_Wo"], f32)).astype(f16),
            "wo2": chunk2(np.asarray(inp["b2_Wo"], f32)).astype(f16),
            "g1_bc": bc(inp["b1_lng"]), "b1_bc": bc(inp["b1_lnb"]),
            "g2_bc": bc(inp["b2_lng"]), "b2_bc": bc(inp["b2_lnb"]),
            "w16": w16,
            "ch_col": ch_col,
            "wv3_s16": chunk2(np.asarray(inp["b3_Wv"], f32)
                              [:, 64 * p:64 * p + 64]).astype(f16),
            "bv3_bc": np.broadcast_to(
                np.asarray(inp["b3_bv"], f32)[64 * p:64 * p + 64],
                (128, 64)).copy(),
            "gam3_col": np.broadcast_to(gam["b3"][heads], (128, 2)).copy(),
            # block3 ctx is gathered per head-pair half (heads 0,2,4,6 then
            # 1,3,5,7) -> permute Wo3 input rows to match
            "wo3": chunk2(np.asarray(inp["b3_Wo"], f32)[
                [32 * h + k for h in (0, 2, 4, 6, 1, 3, 5, 7)
                 for k in range(32)], :]),
            "res3_bc": bc(know + np.asarray(inp["b3_bo"], f32)),
            "g3_bc": bc(inp["b3_lng"]), "b3_bc": bc(inp["b3_lnb"]),
            "lvw_pad16": lvw_pad,
            "lvb_bc": bc(inp["lv_b"]),
            "qrT": np.ascontiguousarray(
                q_emb[b, 512 * p:512 * p + 512].T
                .reshape(2, 128, 512).transpose(1, 0, 2)),
            "kpT": np.ascontiguousarray(know.reshape(8, 32).T),
            "lkw": np.asarray(inp["lk_W"], f32),
            "lkb_col": np.ascontiguousarray(
                np.asarray(inp["lk_b"], f32).reshape(2, 128).T),
            "qsel_col": qsel,
        }
        maps.append(m)
    return maps


# --------------------------------------------------------------------------
# bass program
# --------------------------------------------------------------------------

def _ln(nc, pool, v, g_sb, b_sb, out, tag, eps):
    """out = layernorm(v) * g + b, v fp32 [128, 256]."""
    sv = pool.tile([128, 1], dt.float32, tag=f"sv{tag}")
    nc.vector.tensor_reduce(sv[:], v[:], axis=AX.X, op=ALU.add)
    sq = pool.tile([128, 256], dt.float32, tag=f"sq{tag}")
    s2v = pool.tile([128, 1], dt.float32, tag=f"s2v{tag}")
    nc.scalar.activation(sq[:], v[:], AF.Square, bias=0.0, scale=1.0,
                         accum_out=s2v[:])
    mu = pool.tile([128, 1], dt.float32, tag=f"mu{tag}")
    nc.vector.tensor_scalar(mu[:], sv[:], 1.0 / 256, None, op0=ALU.mult)
    mu2 = pool.tile([128, 1], dt.float32, tag=f"mu2{tag}")
    nc.vector.tensor_tensor(mu2[:], mu[:], mu[:], op=ALU.mult)
    var = pool.tile([128, 1], dt.float32, tag=f"var{tag}")
    nc.vector.scalar_tensor_tensor(var[:], s2v[:], 1.0 / 256, mu2[:],
                                   op0=ALU.mult, op1=ALU.subtract)
    sd = pool.tile([128, 1], dt.float32, tag=f"sd{tag}")
    nc.scalar.activation(sd[:], var[:], AF.Sqrt, bias=eps[:], scale=1.0)
    rstd = pool.tile([128, 1], dt.float32, tag=f"rstd{tag}")
    nc.vector.reciprocal(rstd[:], sd[:])
    xn = pool.tile([128, 256], dt.float32, tag=f"xn{tag}")
    nc.vector.tensor_scalar(xn[:], v[:], mu[:], rstd[:],
                            op0=ALU.subtract, op1=ALU.mult)
    nc.vector.tensor_tensor(xn[:], xn[:], g_sb[:], op=ALU.mult)
    nc.vector.tensor_tensor(out, xn[:], b_sb[:], op=ALU.add)


def _attn_triangle(nc, wp, ctxps, sc_mm, gam_ap, Vtile, v_off,
                   pos, tri, neg16, agdst, agrow, strict, sbc=None,
                   Pbc=None, ninvP=None, ones16=None, grp=GRP):
    """The per-unit decay-bias attention triangle (16 row tiles, grouped).

    Blocks 1/2 path: sc_mm(t, f, fw, tag) emits the scores matmul chunk
    into PSUM and returns the psum tile (stage A reads it through Exp,
    stage D re-issues it for the s2 product — cheaper than keeping an
    fp16 score copy in SBUF). Block3 path (strict=True, sbc/Pbc given):
    rank-1 scores, no matmul/scan. tail/t1 are sums/ratios of
    exponentials so the reference's max(.,0) clamps are dead — plain
    tensor_tensor mult gets the 2x fp16 DVE mode.
    """
    for g0 in range(0, NT, grp):
        tiles = list(range(g0, min(g0 + grp, NT)))
        chain = {}
        if not strict:
            # stage A: scores -> e (Exp); 1/sqrt(dk) is folded into Wq
            for t in tiles:
                W = 128 * (t + 1)
                e = wp.tile([128, 2048], dt.float16, tag=f"ch{t - g0}", name="e")
                chain[t] = e
                for f in range((W + 511) // 512):
                    fw = min(512, W - 512 * f)
                    ps = sc_mm(t, f, fw, "sc")
                    nc.scalar.activation(e[:, 512 * f:512 * f + fw],
                                         ps[:, :fw], AF.Exp, bias=0.0,
                                         scale=1.0)
                nc.vector.copy_predicated(e[:, W - 128:W], tri[:], neg16[1])
            # stage B: suffix scan -> tail, u = tail*pos
            invZ = wp.tile([128, grp], dt.float32, tag="invZ")
            usb = {}
            for t in tiles:
                W = 128 * (t + 1)
                suf = wp.tile([128, 2049], dt.float16, tag=f"sf{t - g0}", name="suf")
                nc.vector.memset(suf[:, W:W + 1], 0.0)
                nc.vector.tensor_tensor_scan(
                    suf[:, :W][:, ::-1], chain[t][:, :W][:, ::-1],
                    chain[t][:, :W][:, ::-1], 0.0, op0=ALU.add, op1=ALU.bypass)
                nc.vector.reciprocal(invZ[:, t - g0:t - g0 + 1], suf[:, 0:1])
                u = wp.tile([128, 2048], dt.float16, tag=f"ch{t - g0}", name="u")
                nc.vector.tensor_tensor(
                    u[:, :W], suf[:, 1:W + 1],
                    pos[:, 2048 - 128 * t:2048 - 128 * t + W], op=ALU.mult)
                usb[t] = u
        else:
            # block3 stage B': t1 = P*(-1/Pprev) + 1, u = t1*pos
            usb = {}
            invZ = None
            for t in tiles:
                W = 128 * (t + 1)
                t1 = wp.tile([128, 2048], dt.float16, tag=f"sf{t - g0}", name="t1")
                nc.vector.scalar_tensor_tensor(
                    t1[:, :W], Pbc[:, :W], ninvP[:, t:t + 1], ones16[:, :W],
                    op0=ALU.mult, op1=ALU.add)
                u = wp.tile([128, 2048], dt.float16, tag=f"ch{t - g0}", name="u")
                nc.vector.tensor_tensor(
                    u[:, :W], t1[:, :W],
                    pos[:, 2048 - 128 * t:2048 - 128 * t + W], op=ALU.mult)
                usb[t] = u
        # stage C: r = sqrt(u * invZ)  (invZ=1 for block3)
        rsb = {}
        for t in tiles:
            W = 128 * (t + 1)
            r = wp.tile([128, 2048], dt.float16, tag=f"sf{t - g0}", name="r")
            if strict:
                nc.scalar.activation(r[:, :W], usb[t][:, :W], AF.Sqrt,
                                     bias=0.0, scale=1.0)
            else:
                nc.scalar.activation(r[:, :W], usb[t][:, :W], AF.Sqrt,
                                     bias=0.0, scale=invZ[:, t - g0:t - g0 + 1])
            rsb[t] = r
        # stage D: eff, scores2, e2, p2, transpose, ctx matmul, store
        for t in tiles:
            W = 128 * (t + 1)
            m0 = wp.tile([128, 2048], dt.float16, tag=f"ch{t - g0}", name="m0")
            nc.scalar.activation(m0[:, :W], rsb[t][:, :W], AF.Exp,
                                 bias=0.0, scale=gam_ap)
            s2 = wp.tile([128, 2048], dt.float16, tag=f"sf{t - g0}", name="s2")
            if strict:
                nc.vector.tensor_tensor(s2[:, :W], m0[:, :W], sbc[:, :W],
                                        op=ALU.mult)
            else:
                for f in range((W + 511) // 512):
                    fw = min(512, W - 512 * f)
                    ps = sc_mm(t, f, fw, "sc2")
                    nc.vector.tensor_tensor(
                        s2[:, 512 * f:512 * f + fw],
                        m0[:, 512 * f:512 * f + fw], ps[:, :fw], op=ALU.mult)
            nc.vector.copy_predicated(s2[:, W - 128:W], tri[:], neg16[0])
            e2 = wp.tile([128, 2048], dt.float16, tag=f"ch{t - g0}", name="e2")
            Z2 = wp.tile([128, 1], dt.float32, tag="Z2")
            nc.scalar.activation(e2[:, :W], s2[:, :W], AF.Exp, bias=0.0,
                                 scale=1.0, accum_out=Z2[:])
            iZ2 = wp.tile([128, 1], dt.float32, tag="iZ2")
            nc.vector.reciprocal(iZ2[:], Z2[:])
            p2 = wp.tile([128, 2048], dt.float16, tag=f"sf{t - g0}", name="p2")
            nc.vector.tensor_scalar(p2[:, :W], e2[:, :W], iZ2[:], None,
                                    op0=ALU.mult)
            if strict and t == 0:
                nc.vector.memset(p2[0:1, 0:128], 0.0)
            p2T = wp.tile([128, 16, 128], dt.float16, tag=f"ch{t - g0}", name="p2T")
            nc.sync.dma_start_transpose(p2T[:, :t + 1, :], p2[:, :W])
            cps = ctxps.tile([32, 128], dt.float32, tag="ctx")
            for jb in range(t + 1):
                nc.tensor.matmul(cps[:], Vtile[:, jb, v_off:v_off + 32],
                                 p2T[:, jb, :], start=(jb == 0), stop=(jb == t))
            csb = wp.tile([32, 128], dt.float32, tag="csb", name="csb")
            nc.vector.tensor_copy(csb[:], cps[:])
            nc.sync.dma_start(
                agdst[agrow:agrow + 32, 128 * t:128 * t + 128], csb[:])


def build_bass(sim=False):
    nc = bacc.Bacc("TRN2", target_bir_lowering=False)

    def din(name, shape, dtyp=dt.float32):
        return nc.dram_tensor(name, shape, dtyp, kind="ExternalInput")

    tin = {
        "xT": din("xT", [128, 2, 2, 2048], dt.float32r),
        "wq_s": din("wq_s", [128, 2, 2, 2, 32], dt.float32r),
        "bq_col": din("bq_col", [32, 2, 2]),
        "wv_s": din("wv_s", [128, 2, 2, 64], dt.float32r),
        "bv_bc": din("bv_bc", [128, 2, 64]),
        "gam_col": din("gam_col", [128, 2, 2]),
        "pos16": din("pos16", [128, 4096], dt.float16),
        "tri_pk": din("tri_pk", [128, 128], dt.uint8),
        "tri_st": din("tri_st", [128, 128], dt.uint8),
        "r1q": din("r1q", [512, 256]),
        "r2q": din("r2q", [512, 256]),
        "wo1": din("wo1", [128, 2, 256], dt.float16),
        "wo2": din("wo2", [128, 2, 256], dt.float16),
        "g1_bc": din("g1_bc", [128, 256]), "b1_bc": din("b1_bc", [128, 256]),
        "g2_bc": din("g2_bc", [128, 256]), "b2_bc": din("b2_bc", [128, 256]),
        "w16": din("w16", [128, 2, 2], dt.float16),
        "ch_col": din("ch_col", [1, 2]),
        "wv3_s16": din("wv3_s16", [128, 2, 64], dt.float16),
        "bv3_bc": din("bv3_bc", [128, 64]),
        "gam3_col": din("gam3_col", [128, 2]),
        "wo3": din("wo3", [128, 2, 256]),
        "res3_bc": din("res3_bc", [128, 256]),
        "g3_bc": din("g3_bc", [128, 256]), "b3_bc": din("b3_bc", [128, 256]),
        "lvw_pad16": din("lvw_pad16", [128, 8, 256], dt.float16),
        "lvb_bc": din("lvb_bc", [128, 256]),
        "qrT": din("qrT", [128, 2, 512]),
        "kpT": din("kpT", [32, 8]),
        "lkw": din("lkw", [32, 256]),
        "lkb_col": din("lkb_col", [128, 2]),
        "qsel_col": din("qsel_col", [128, 4]),
    }
    out_t = nc.dram_tensor("out", [512, 256], dt.float32,
                           kind="ExternalOutput")

    with tile.TileContext(nc) as tc, \
         tc.tile_pool(name="consts", bufs=1) as cs, \
         tc.tile_pool(name="dram", bufs=1, space="DRAM") as dram:
        pos = cs.tile([128, 4096], dt.float16)
        nc.sync.dma_start(pos[:], tin["pos16"][:])
        tpk = cs.tile([128, 128], dt.uint8)
        nc.sync.dma_start(tpk[:], tin["tri_pk"][:])
        tst = cs.tile([128, 128], dt.uint8)
        nc.sync.dma_start(tst[:], tin["tri_st"][:])
        gamc = cs.tile([128, 2, 2], dt.float32)
        nc.sync.dma_start(gamc[:], tin["gam_col"][:])
        gam3c = cs.tile([128, 2], dt.float32)
        nc.sync.dma_start(gam3c[:], tin["gam3_col"][:])
        bqc = cs.tile([32, 2, 2], dt.float32)
        nc.sync.dma_start(bqc[:], tin["bq_col"][:])
        bvb = cs.tile([128, 2, 64], dt.float32)
        nc.sync.dma_start(bvb[:], tin["bv_bc"][:])
        neg16 = cs.tile([128, 128], dt.float16)
        nc.vector.memset(neg16[:], NEG)
        zero16 = cs.tile([128, 128], dt.float16)
        nc.vector.memset(zero16[:], 0.0)
        ones16 = cs.tile([128, 2048], dt.float16)
        nc.vector.memset(ones16[:], 1.0)
        tch = cs.tile([1, 4], dt.float16)
        nc.vector.tensor_copy(tch[:, 0:1], pos[:1, :1])
        epsc = cs.tile([128, 1], dt.float32)
        nc.vector.memset(epsc[:], 1e-5)

        # per-block / per-head-pair gather halves so each collective can
        # overlap the next unit's compute instead of stalling the core
        agin1a = dram.tile([64, 2048], dt.float32)
        agout1a = dram.tile([256, 2048], dt.float32)
        agin1b = dram.tile([64, 2048], dt.float32)
        agout1b = dram.tile([256, 2048], dt.float32)
        agin2a = dram.tile([32, 2048], dt.float32)
        agout2a = dram.tile([128, 2048], dt.float32)
        agin2b = dram.tile([32, 2048], dt.float32)
        agout2b = dram.tile([128, 2048], dt.float32)
        agin3 = dram.tile([128, 2048], dt.float16)
        agout3 = dram.tile([512, 2048], dt.float16)
        pbuf = dram.tile([2, 2049], dt.float32)

        # ---------------- phase 1 ----------------
        qtv_cm = tc.tile_pool(name="qtv", bufs=1)
        qtv = qtv_cm.__enter__()
        QT = qtv.tile([32, 2, 2, 2048], dt.float16)
        V16 = qtv.tile([128, 2, 16, 64], dt.float16)
        with tc.tile_pool(name="proj", bufs=1) as pj, \
             tc.tile_pool(name="pjps", bufs=2, space="PSUM") as pjps:
            xTs = pj.tile([128, 2, 2, 2048], dt.float32r)
            nc.sync.dma_start(xTs[:], tin["xT"][:])
            wqs = pj.tile([128, 2, 2, 2, 32], dt.float32r)
            nc.sync.dma_start(wqs[:], tin["wq_s"][:])
            wvs = pj.tile([128, 2, 2, 64], dt.float32r)
            nc.sync.dma_start(wvs[:], tin["wv_s"][:])
            for blk in range(2):
                for hi in range(2):
                    for f in range(4):
                        ps = pjps.tile([32, 512], dt.float32, tag="qt")
                        for cch in range(2):
                            nc.tensor.matmul(
                                ps[:], wqs[:, cch, blk, hi, :],
                                xTs[:, cch, blk, 512 * f:512 * f + 512],
                                start=(cch == 0), stop=(cch == 1))
                        nc.scalar.activation(
                            QT[:, blk, hi, 512 * f:512 * f + 512], ps[:],
                            AF.Identity, bias=bqc[:, blk, hi:hi + 1],
                            scale=1.0)
                for jb in range(16):
                    ps = pjps.tile([128, 64], dt.float32, tag="v")
                    for cch in range(2):
                        nc.tensor.matmul(
                            ps[:], xTs[:, cch, blk, 128 * jb:128 * jb + 128],
                            wvs[:, cch, blk, :], start=(cch == 0),
                            stop=(cch == 1))
                    nc.vector.tensor_tensor(V16[:, blk, jb, :], ps[:],
                                            bvb[:, blk, :], op=ALU.add)

        with tc.tile_pool(name="p1", bufs=1) as wp, \
             tc.tile_pool(name="scps", bufs=3, space="PSUM") as scps, \
             tc.tile_pool(name="ctxps", bufs=2, space="PSUM") as ctxps:
            for blk in range(2):
                agdst = (agin1a, agin1b)[blk]
                for hi in range(2):
                    def sc_mm(t, f, fw, tag, blk=blk, hi=hi):
                        ps = scps.tile([128, 512], dt.float32, tag=tag)
                        nc.tensor.matmul(
                            ps[:, :fw],
                            QT[:, blk, hi, 128 * t:128 * t + 128],
                            QT[:, blk, hi, 512 * f:512 * f + fw],
                            start=True, stop=True)
                        return ps
                    _attn_triangle(
                        nc, wp, ctxps, sc_mm,
                        gamc[:, blk, hi:hi + 1], V16[:, blk], 32 * hi,
                        pos, tpk, (neg16, zero16), agdst,
                        32 * hi, strict=False)
                # gather this block's context now; overlaps the next
                # block's triangles / the hq rebuild
                agout = (agout1a, agout1b)[blk]
                if sim:
                    for rr in range(4):
                        nc.gpsimd.dma_start(agout[64 * rr:64 * rr + 64, :],
                                            agdst[:])
                else:
                    nc.gpsimd.collective_compute(
                        "AllGather", ALU.bypass, ins=[agdst.opt()],
                        outs=[agout.opt()],
                        replica_groups=[[0, 1, 2, 3], [4, 5, 6, 7]])

        qtv_cm.__exit__(None, None, None)

        # ---------------- phase 2: hq/ha ----------------
        hq16 = cs.tile([128, 2, 2048], dt.float16)
        ha16 = cs.tile([128, 2, 2048], dt.float16)
        with tc.tile_pool(name="p2", bufs=2) as p2, \
             tc.tile_pool(name="lnp", bufs=4) as lnp, \
             tc.tile_pool(name="p2ps", bufs=3, space="PSUM") as p2ps:
            # each core rebuilds only its 512-row quarter (one-hot column
            # blend picks the quarter), then an fp16 AllGather reassembles
            # the transposed hq/ha for block3
            qsel2 = p2.tile([128, 4], dt.float32, tag="qsel2")
            nc.sync.dma_start(qsel2[:], tin["qsel_col"][:])
            for which, (wo_n, res_n, g_n, bb_n, agsrc) in enumerate([
                    ("wo1", "r1q", "g1_bc", "b1_bc", agout1a),
                    ("wo2", "r2q", "g2_bc", "b2_bc", agout1b)]):
                wo_sb = p2.tile([128, 2, 256], dt.float16, tag="wo")
                nc.sync.dma_start(wo_sb[:], tin[wo_n][:])
                g_sb = p2.tile([128, 256], dt.float32, tag="g")
                nc.sync.dma_start(g_sb[:], tin[g_n][:])
                bb_sb = p2.tile([128, 256], dt.float32, tag="bb")
                nc.sync.dma_start(bb_sb[:], tin[bb_n][:])
                ctxT = p2.tile([128, 2, 2048], dt.float32, tag="ctxT")
                for cch in range(2):
                    nc.sync.dma_start(
                        ctxT[:, cch, :],
                        agsrc[128 * cch:128 * cch + 128, :])
                ctxq = p2.tile([128, 2, 512], dt.float16, tag="ctxq")
                for cch in range(2):
                    blendt = p2.tile([128, 512], dt.float32, tag="blend2")
                    nc.vector.tensor_scalar(
                        blendt[:], ctxT[:, cch, 0:512], qsel2[:, 0:1], None,
                        op0=ALU.mult)
                    for qq in range(1, 4):
                        dst = ctxq[:, cch, :] if qq == 3 else blendt[:]
                        nc.vector.scalar_tensor_tensor(
                            dst, ctxT[:, cch, 512 * qq:512 * qq + 512],
                            qsel2[:, qq:qq + 1], blendt[:],
                            op0=ALU.mult, op1=ALU.add)
                h16 = p2.tile([128, 4, 256], dt.float16, tag="h16")
                hTq = p2.tile([128, 2, 512], dt.float16, tag="hTq")
                for ic in range(4):
                    ps = p2ps.tile([128, 256], dt.float32, tag="wops")
                    for cch in range(2):
                        nc.tensor.matmul(
                            ps[:], ctxq[:, cch, 128 * ic:128 * ic + 128],
                            wo_sb[:, cch, :], start=(cch == 0),
                            stop=(cch == 1))
                    res_sb = lnp.tile([128, 256], dt.float32, tag="res")
                    nc.sync.dma_start(res_sb[:],
                                      tin[res_n][128 * ic:128 * ic + 128, :])
                    v = lnp.tile([128, 256], dt.float32, tag="v")
                    nc.vector.tensor_tensor(v[:], ps[:], res_sb[:], op=ALU.add)
                    _ln(nc, lnp, v, g_sb, bb_sb, h16[:, ic, :], "a", epsc)
                    nc.sync.dma_start_transpose(
                        hTq[:, :, 128 * ic:128 * ic + 128], h16[:, ic, :])
                nc.sync.dma_start(agin3[:, 1024 * which:1024 * which + 1024],
                                  hTq[:])
            if sim:
                for rr in range(4):
                    nc.gpsimd.dma_start(
                        agout3[128 * rr:128 * rr + 128, :], agin3[:])
            else:
                nc.gpsimd.collective_compute(
                    "AllGather", ALU.bypass, ins=[agin3.opt()],
                    outs=[agout3.opt()],
                    replica_groups=[[0, 1, 2, 3], [4, 5, 6, 7]])
            for cc in range(4):
                for cch in range(2):
                    nc.sync.dma_start(
                        hq16[:, cch, 512 * cc:512 * cc + 512],
                        agout3[128 * cc:128 * cc + 128,
                               512 * cch:512 * cch + 512])
                    nc.sync.dma_start(
                        ha16[:, cch, 512 * cc:512 * cc + 512],
                        agout3[128 * cc:128 * cc + 128,
                               1024 + 512 * cch:1024 + 512 * cch + 512])

        # ---------------- phase 2: block3 ----------------
        with tc.tile_pool(name="b3", bufs=1) as wp, \
             tc.tile_pool(name="b3ps", bufs=2, space="PSUM") as ps3, \
             tc.tile_pool(name="b3ctx", bufs=2, space="PSUM") as ctxps:
            w16sb = wp.tile([128, 2, 2], dt.float16, tag="w16")
            nc.sync.dma_start(w16sb[:], tin["w16"][:])
            chc = wp.tile([1, 2], dt.float32, tag="chc")
            nc.sync.dma_start(chc[:], tin["ch_col"][:])
            wv3 = wp.tile([128, 2, 64], dt.float16, tag="wv3")
            nc.sync.dma_start(wv3[:], tin["wv3_s16"][:])
            bv3 = wp.tile([128, 64], dt.float32, tag="bv3")
            nc.sync.dma_start(bv3[:], tin["bv3_bc"][:])
            one1 = wp.tile([1, 1], dt.float32, tag="one1")
            nc.vector.memset(one1[:], 1.0)
            onesrow = wp.tile([1, 128], dt.float32, tag="onesrow")
            nc.vector.memset(onesrow[:], 1.0)

            V3 = wp.tile([128, 16, 64], dt.float16, tag="V3")
            for jb in range(16):
                vps = ps3.tile([128, 64], dt.float32, tag="v3")
                for cch in range(2):
                    nc.tensor.matmul(
                        vps[:], ha16[:, cch, 128 * jb:128 * jb + 128],
                        wv3[:, cch, :], start=(cch == 0), stop=(cch == 1))
                nc.vector.tensor_tensor(V3[:, jb, :], vps[:], bv3[:],
                                        op=ALU.add)

            for hi in range(2):
                s_row = wp.tile([1, 2048], dt.float32, tag="srow")
                for f in range(4):
                    sp = ps3.tile([1, 512], dt.float32, tag="s")
                    for cch in range(2):
                        nc.tensor.matmul(
                            sp[:], w16sb[:, cch, hi:hi + 1],
                            hq16[:, cch, 512 * f:512 * f + 512],
                            start=(cch == 0), stop=(cch == 1))
                    nc.scalar.activation(
                        s_row[:, 512 * f:512 * f + 512], sp[:], AF.Identity,
                        bias=chc[:, hi:hi + 1], scale=ISQ)
                smax = wp.tile([1, 1], dt.float32, tag="smax")
                nc.vector.tensor_reduce(smax[:], s_row[:], axis=AX.X,
                                        op=ALU.max)
                nsmax = wp.tile([1, 1], dt.float32, tag="nsmax")
                nc.vector.tensor_scalar(nsmax[:], smax[:], -1.0, None,
                                        op0=ALU.mult)
                e3 = wp.tile([1, 2048], dt.float32, tag="e3")
                nc.scalar.activation(e3[:], s_row[:], AF.Exp, bias=nsmax[:],
                                     scale=1.0)
                P_row = wp.tile([1, 2048], dt.float32, tag="Prow")
                nc.vector.tensor_tensor_scan(P_row[:], e3[:], e3[:], 0.0,
                                             op0=ALU.add, op1=ALU.bypass)
                nc.sync.dma_start(pbuf[hi, 0:1], one1[:])
                nc.sync.dma_start(pbuf[hi, 1:2049], P_row[:])
                npcol = wp.tile([128, 16], dt.float32, tag="npcol")
                pcol = wp.tile([128, 16], dt.float32, tag="pcol")
                nc.sync.dma_start(
                    pcol[:], pbuf[hi, 0:2048].rearrange("(t p) -> p t", p=128))
                nc.vector.tensor_scalar(pcol[:], pcol[:], -1.0, None,
                                        op0=ALU.mult)
                nc.vector.reciprocal(npcol[:], pcol[:])
                P_bc = wp.tile([128, 2048], dt.float32, tag="Pbc")
                s_bc = wp.tile([128, 2048], dt.float16, tag="sbc")
                for f in range(4):
                    bp = ps3.tile([128, 512], dt.float32, tag="bc")
                    nc.tensor.matmul(bp[:], onesrow[:],
                                     P_row[:, 512 * f:512 * f + 512],
                                     start=True, stop=True)
                    nc.vector.tensor_copy(P_bc[:, 512 * f:512 * f + 512],
                                          bp[:])
                    bs = ps3.tile([128, 512], dt.float32, tag="bc")
                    nc.tensor.matmul(bs[:], onesrow[:],
                                     s_row[:, 512 * f:512 * f + 512],
                                     start=True, stop=True)
                    nc.vector.tensor_copy(s_bc[:, 512 * f:512 * f + 512],
                                          bs[:])
                agdst = (agin2a, agin2b)[hi]
                _attn_triangle(
                    nc, wp, ctxps, None, gam3c[:, hi:hi + 1],
                    V3, 32 * hi, pos, tst, (neg16, zero16), agdst, 0,
                    strict=True, sbc=s_bc, Pbc=P_bc, ninvP=npcol,
                    ones16=ones16)
                agout = (agout2a, agout2b)[hi]
                if sim:
                    for rr in range(4):
                        nc.gpsimd.dma_start(
                            agout[32 * rr:32 * rr + 32, :], agdst[:])
                else:
                    nc.gpsimd.collective_compute(
                        "AllGather", ALU.bypass, ins=[agdst.opt()],
                        outs=[agout.opt()],
                        replica_groups=[[0, 1, 2, 3], [4, 5, 6, 7]])

        # ---------------- phase 3 ----------------
        with tc.tile_pool(name="p3", bufs=2) as wp, \
             tc.tile_pool(name="p3ps", bufs=2, space="PSUM") as ps:
            wo3 = wp.tile([128, 2, 256], dt.float32, tag="wo3")
            nc.sync.dma_start(wo3[:], tin["wo3"][:])
            res3 = wp.tile([128, 256], dt.float32, tag="res3")
            nc.sync.dma_start(res3[:], tin["res3_bc"][:])
            g3 = wp.tile([128, 256], dt.float32, tag="g3")
            nc.sync.dma_start(g3[:], tin["g3_bc"][:])
            b3 = wp.tile([128, 256], dt.float32, tag="b3")
            nc.sync.dma_start(b3[:], tin["b3_bc"][:])
            lvw = wp.tile([128, 8, 256], dt.float16, tag="lvw")
            nc.sync.dma_start(lvw[:], tin["lvw_pad16"][:])
            lvb = wp.tile([128, 256], dt.float32, tag="lvb")
            nc.sync.dma_start(lvb[:], tin["lvb_bc"][:])
            qrTs = wp.tile([128, 2, 512], dt.float32, tag="qrTs")
            nc.sync.dma_start(qrTs[:], tin["qrT"][:])
            kpTs = wp.tile([32, 8], dt.float32, tag="kpTs")
            nc.sync.dma_start(kpTs[:], tin["kpT"][:])
            lkws = wp.tile([32, 256], dt.float32, tag="lkws")
            nc.sync.dma_start(lkws[:], tin["lkw"][:])
            lkbc = wp.tile([128, 2], dt.float32, tag="lkbc")
            nc.sync.dma_start(lkbc[:], tin["lkb_col"][:])
            qsel = wp.tile([128, 4], dt.float32, tag="qsel")
            nc.sync.dma_start(qsel[:], tin["qsel_col"][:])

            keyT = wp.tile([128, 2, 8], dt.float32, tag="keyT")
            for cch in range(2):
                kps = ps.tile([128, 8], dt.float32, tag="key")
                nc.tensor.matmul(kps[:], lkws[:, 128 * cch:128 * cch + 128],
                                 kpTs[:], start=True, stop=True)
                nc.scalar.activation(keyT[:, cch, :], kps[:], AF.Sigmoid,
                                     bias=lkbc[:, cch:cch + 1], scale=1.0)

            ag2f = wp.tile([128, 2, 2048], dt.float32, tag="ag2f")
            for cch, agsrc in enumerate((agout2a, agout2b)):
                nc.sync.dma_start(ag2f[:, cch, :], agsrc[:])
            # select this core's column quarter via the one-hot qsel blend
            ag2sb = wp.tile([128, 2, 512], dt.float32, tag="ag2sb")
            for cch in range(2):
                blendt = wp.tile([128, 512], dt.float32, tag="blendt")
                nc.vector.tensor_scalar(
                    blendt[:], ag2f[:, cch, 0:512], qsel[:, 0:1], None,
                    op0=ALU.mult)
                for qq in range(1, 4):
                    dst = blendt if qq < 3 else None
                    if qq < 3:
                        nc.vector.scalar_tensor_tensor(
                            blendt[:], ag2f[:, cch, 512 * qq:512 * qq + 512],
                            qsel[:, qq:qq + 1], blendt[:],
                            op0=ALU.mult, op1=ALU.add)
                    else:
                        nc.vector.scalar_tensor_tensor(
                            ag2sb[:, cch, :],
                            ag2f[:, cch, 512 * qq:512 * qq + 512],
                            qsel[:, qq:qq + 1], blendt[:],
                            op0=ALU.mult, op1=ALU.add)

            # stage-major over the 4 row tiles so ACT runs each function
            # (sqrt-table LN, Exp, Sigmoid) as one batch - 3 table loads
            # instead of ~4 per tile
            h3Ts, alphas = [], []
            for ic in range(4):
                wops = ps.tile([128, 256], dt.float32, tag="wo3ps")
                for cch in range(2):
                    nc.tensor.matmul(
                        wops[:], ag2sb[:, cch, 128 * ic:128 * ic + 128],
                        wo3[:, cch, :], start=(cch == 0), stop=(cch == 1))
                v = wp.tile([128, 256], dt.float32, tag=f"v3p{ic}")
                nc.vector.tensor_tensor(v[:], wops[:], res3[:], op=ALU.add)
                h3 = wp.tile([128, 256], dt.float32, tag=f"h3{ic}")
                _ln(nc, wp, v, g3, b3, h3[:], f"3{ic}", epsc)
                h316 = wp.tile([128, 256], dt.float16, tag=f"h316{ic}")
                nc.vector.tensor_copy(h316[:], h3[:])
                h3T = wp.tile([128, 2, 128], dt.float16, tag=f"h3T{ic}")
                nc.sync.dma_start_transpose(h3T[:], h316[:])
                h3Ts.append(h3T)
            for ic in range(4):
                bps = ps.tile([128, 8], dt.float32, tag="beta")
                for cch in range(2):
                    nc.tensor.matmul(
                        bps[:], qrTs[:, cch, 128 * ic:128 * ic + 128],
                        keyT[:, cch, :], start=(cch == 0), stop=(cch == 1))
                bmax = wp.tile([128, 1], dt.float32, tag=f"bmax{ic}")
                nc.vector.tensor_reduce(bmax[:], bps[:], axis=AX.X,
                                        op=ALU.max)
                nbmax = wp.tile([128, 1], dt.float32, tag=f"nbmax{ic}")
                nc.vector.tensor_scalar(nbmax[:], bmax[:], -1.0, None,
                                        op0=ALU.mult)
                ebeta = wp.tile([128, 8], dt.float32, tag=f"ebeta{ic}")
                zb = wp.tile([128, 1], dt.float32, tag=f"zb{ic}")
                nc.scalar.activation(ebeta[:], bps[:], AF.Exp, bias=nbmax[:],
                                     scale=1.0, accum_out=zb[:])
                izb = wp.tile([128, 1], dt.float32, tag=f"izb{ic}")
                nc.vector.reciprocal(izb[:], zb[:])
                alpha = wp.tile([128, 8], dt.float32, tag=f"alpha{ic}")
                nc.vector.tensor_scalar(alpha[:], ebeta[:], izb[:], None,
                                        op0=ALU.mult)
                alphas.append(alpha)
            for ic in range(4):
                h3T, alpha = h3Ts[ic], alphas[ic]
                acc = wp.tile([128, 256], dt.float32, tag=f"acc{ic}")
                accb = wp.tile([128, 256], dt.float32, tag=f"accb{ic}")
                nc.vector.memset(acc[:], 0.0)
                for h in range(8):
                    vps = ps.tile([128, 256], dt.float32, tag="valps")
                    nc.tensor.matmul(vps[:], h3T[:, h // 4, :], lvw[:, h, :],
                                     start=True, stop=True)
                    val = wp.tile([128, 256], dt.float32, tag="val")
                    nc.vector.tensor_tensor(val[:], vps[:], lvb[:],
                                            op=ALU.add)
                    vsg = wp.tile([128, 256], dt.float32, tag="vsg")
                    nc.scalar.activation(vsg[:], val[:], AF.Sigmoid,
                                         bias=0.0, scale=1.0)
                    src, dst2 = (acc, accb) if h % 2 == 0 else (accb, acc)
                    nc.vector.scalar_tensor_tensor(
                        dst2[:], vsg[:], alpha[:, h:h + 1], src[:],
                        op0=ALU.mult, op1=ALU.add)
                nc.sync.dma_start(out_t[128 * ic:128 * ic + 128, :], acc[:])

    nc.finalize()
    return nc


def run(inputs, **kw):
    if "nc" not in _BUILT:
        _BUILT["nc"] = build_bass()
    nc = _BUILT["nc"]
    in_maps = build_in_maps(inputs)
    res = bass_utils.run_bass_kernel_spmd(nc, in_maps,
                                          core_ids=list(range(8)), **kw)
    out = np.zeros((2, 2048, 256), np.float32)
    for c in range(8):
        b, q = c // 4, c % 4
        out[b, 512 * q:512 * q + 512, :] = res.results[c]["out"]
    return out, res


def kernel(**inputs):
    return run(inputs)[0]

